# revision 20
# baseline (speedup 1.0000x reference)
"""Trainium2 Bass kernel for BoundaryRefinementModule.

One image per NeuronCore (B=8 over 8 cores, pure data parallel).
Convs are 9-shifted-matmuls accumulating in PSUM; activations in bf16,
accumulation + stats + pointwise in fp32.

Layout: activations live in SBUF "canvases" of shape [128, 16904] bf16,
where pixel (h, w) of a 128x128 image sits at free index (h+1)*130+(w+1)
(a 130x130 zero-padded canvas, flattened; +4 tail pad so conv windows
never slice out of range).
"""

import numpy as np
import ml_dtypes

# ---------------- geometry ----------------
H = W = 128
HW = H * W
CW = 130                  # canvas row width
LC = 16904                # canvas free elems (130*130 + 4 tail)
ORG = CW + 1              # canvas index of pixel (0,0)
RT = 4                    # image rows per spatial tile
NT = H // RT              # 32 spatial tiles
NPT = RT * W              # 512 pixels per tile
TAPS = [(ty, tx) for ty in (-1, 0, 1) for tx in (-1, 0, 1)]
LAP_TAPS = [((-1, 0), 1.0), ((0, -1), 1.0), ((0, 0), -4.0), ((0, 1), 1.0), ((1, 0), 1.0)]
EPS_GN = 1e-5

_CACHE = {}


# ---------------- host-side weight packing ----------------
def _pack_conv(w, cin_chunks, cout_chunks, mc):
    """w: (Cout, Cin, 3, 3) fp32 -> (128, 9*cin_chunks*cout_chunks*mc) bf16.

    Column block for (tap t, cin chunk c, cout chunk m):
      col0 = ((t*cin_chunks + c)*cout_chunks + m)*mc
      A[k, col0+j] = w[m*mc+j, c*128+k, ty+1, tx+1]
    """
    ncol = 9 * cin_chunks * cout_chunks * mc
    A = np.zeros((128, ncol), np.float32)
    for t, (ty, tx) in enumerate(TAPS):
        for c in range(cin_chunks):
            for m in range(cout_chunks):
                col0 = ((t * cin_chunks + c) * cout_chunks + m) * mc
                blk = w[m * mc:(m + 1) * mc, c * 128:(c + 1) * 128, ty + 1, tx + 1]
                A[:, col0:col0 + mc] = blk.T
    return A.astype(ml_dtypes.bfloat16)


def _host_consts():
    # Vertical Sobel band matrices (SAME zero boundary), fp32.
    Bs = np.zeros((128, 128), np.float32)   # smoothing [1,2,1]
    Bd = np.zeros((128, 128), np.float32)   # difference [-1,0,1]
    for h in range(128):
        for d in (-1, 0, 1):
            k = h + d
            if 0 <= k < 128:
                Bs[k, h] = (1.0, 2.0, 1.0)[d + 1]
                Bd[k, h] = float(d)
    bsbd = np.concatenate([Bs, Bd], axis=1)  # (128, 256)

    def block_ind(p, g):
        M = np.zeros((p, p), np.float32)
        for i in range(p):
            M[i, (i // g) * g:(i // g + 1) * g] = 1.0
        return M

    gmat16 = block_ind(128, 16)   # hf GN: 128 ch, groups of 16
    gmat32 = block_ind(128, 32)   # ref GN: per-chunk groups of 32
    gmat8 = block_ind(64, 8)      # gate GN: 64 ch, groups of 8

    idlap = np.zeros((128, 256), np.float32)
    idlap[:, 0:128] = np.eye(128)
    idlap[:, 128:256] = -4.0 * np.eye(128)
    ones1 = np.full((128, 1), 1.0 / 256.0, np.float32)  # channel-mean weights
    return {
        "bsbd": bsbd.astype(np.float32),
        "gmat16": gmat16, "gmat32": gmat32, "gmat8": gmat8,
        "idlap": idlap.astype(ml_dtypes.bfloat16),
        "ones1": ones1.astype(np.float32),
    }


# ---------------- bass program ----------------
def _build():
    if "built" in _CACHE:
        return _CACHE["built"]
    from contextlib import ExitStack
    import concourse.bacc as bacc
    import concourse.tile as tile
    from concourse import mybir
    import concourse.bass as bass

    f32 = mybir.dt.float32
    bf16 = mybir.dt.bfloat16
    AF = mybir.ActivationFunctionType
    ALU = mybir.AluOpType
    AX = mybir.AxisListType

    nc = bacc.Bacc("TRN2", target_bir_lowering=False, debug=False, num_devices=8)

    # -------- DRAM I/O --------
    fused = nc.dram_tensor("fused", [256, HW], f32, kind="ExternalInput").ap()
    whf1 = nc.dram_tensor("whf1", [128, 2304], bf16, kind="ExternalInput").ap()
    whf2 = nc.dram_tensor("whf2", [128, 2304], bf16, kind="ExternalInput").ap()
    wg1 = nc.dram_tensor("wg1", [128, 1152], bf16, kind="ExternalInput").ap()
    wg1e = nc.dram_tensor("wg1e", [9, 64], bf16, kind="ExternalInput").ap()
    wg2 = nc.dram_tensor("wg2", [64, 1], bf16, kind="ExternalInput").ap()
    wr1 = nc.dram_tensor("wr1", [128, 4608], bf16, kind="ExternalInput").ap()
    wr2 = nc.dram_tensor("wr2", [128, 4608], bf16, kind="ExternalInput").ap()
    bsbd_d = nc.dram_tensor("bsbd", [128, 256], f32, kind="ExternalInput").ap()
    gmat16_d = nc.dram_tensor("gmat16", [128, 128], f32, kind="ExternalInput").ap()
    gmat32_d = nc.dram_tensor("gmat32", [128, 128], f32, kind="ExternalInput").ap()
    gmat8_d = nc.dram_tensor("gmat8", [64, 64], f32, kind="ExternalInput").ap()
    idlap_d = nc.dram_tensor("idlap", [128, 256], bf16, kind="ExternalInput").ap()
    ones1_d = nc.dram_tensor("ones1", [128, 1], f32, kind="ExternalInput").ap()
    gnwh_d = nc.dram_tensor("gnwh", [128, 1], f32, kind="ExternalInput").ap()
    gnbh_d = nc.dram_tensor("gnbh", [128, 1], f32, kind="ExternalInput").ap()
    gnwg_d = nc.dram_tensor("gnwg", [64, 1], f32, kind="ExternalInput").ap()
    gnbg_d = nc.dram_tensor("gnbg", [64, 1], f32, kind="ExternalInput").ap()
    gnwr_d = nc.dram_tensor("gnwr", [128, 2], f32, kind="ExternalInput").ap()
    gnbr_d = nc.dram_tensor("gnbr", [128, 2], f32, kind="ExternalInput").ap()
    b2s_d = nc.dram_tensor("b2s", [1, 1], f32, kind="ExternalInput").ap()
    rss_d = nc.dram_tensor("rss", [1, 1], f32, kind="ExternalInput").ap()

    refined = nc.dram_tensor("refined", [256, HW], f32, kind="ExternalOutput").ap()
    edge_o = nc.dram_tensor("edge_o", [128, 128], f32, kind="ExternalOutput").ap()
    gate_o = nc.dram_tensor("gate_o", [1, HW], f32, kind="ExternalOutput").ap()



    def win(cv, ti, off, parts=None):
        """Conv rhs / write window: (P, RT, 128), canvas row stride CW."""
        base = (ti * RT + 1) * CW + 1 + off
        w = cv[:, base:base + RT * CW].rearrange("p (r x) -> p r x", r=RT)
        return w[:, :, 0:W]

    def interior(cv):
        w = cv[:, CW:CW + H * CW].rearrange("p (r x) -> p r x", r=H)
        return w[:, :, 1:1 + W]

    with tile.TileContext(nc) as tc, ExitStack() as CTX:
        # ---------------- persistent pools ----------------
        pw = CTX.enter_context(tc.tile_pool(name="pw", bufs=1))
        pstat = CTX.enter_context(tc.tile_pool(name="pstat", bufs=1))
        pstage = CTX.enter_context(tc.tile_pool(name="pstage", bufs=1))
        pps = CTX.enter_context(tc.tile_pool(name="pps", bufs=6, space="PSUM"))
        ppsm = CTX.enter_context(tc.tile_pool(name="ppsm", bufs=2, space="PSUM"))
        pdram = CTX.enter_context(tc.tile_pool(name="pdram", bufs=1, space="DRAM"))

        # DRAM spill tiles (tracked by Tile for DMA ordering)
        hfsp = [pdram.tile([128, LC], bf16, name=f"hfsp{c}", tag=f"hfsp{c}") for c in range(2)]
        fzlin = [pdram.tile([128, HW], bf16, name=f"fzlin{c}", tag=f"fzlin{c}") for c in range(2)]
        eflat_sp = pdram.tile([1, HW], bf16, name="eflat_sp", tag="eflat_sp")
        gspill = pdram.tile([1, HW], bf16, name="gspill", tag="gspill")

        # weights / consts to SBUF
        def load(name, src, shape, dt):
            t = pw.tile(shape, dt, name=name, tag=name)
            nc.sync.dma_start(t[:], src)
            return t

        whf1_s = load("whf1s", whf1, [128, 2304], bf16)
        whf2_s = load("whf2s", whf2, [128, 2304], bf16)
        wg1_s = load("wg1s", wg1, [128, 1152], bf16)
        wg1e_s = load("wg1es", wg1e, [9, 64], bf16)
        wg2_s = load("wg2s", wg2, [64, 1], bf16)
        wr1_s = load("wr1s", wr1, [128, 4608], bf16)
        wr2_s = load("wr2s", wr2, [128, 4608], bf16)
        bsbd_s = load("bsbds", bsbd_d, [128, 256], f32)
        gmat16_s = load("gmat16s", gmat16_d, [128, 128], f32)
        gmat32_s = load("gmat32s", gmat32_d, [128, 128], f32)
        gmat8_s = load("gmat8s", gmat8_d, [64, 64], f32)
        idlap_s = load("idlaps", idlap_d, [128, 256], bf16)
        ones1_s = load("ones1s", ones1_d, [128, 1], f32)
        gnwh_s = load("gnwhs", gnwh_d, [128, 1], f32)
        gnbh_s = load("gnbhs", gnbh_d, [128, 1], f32)
        gnwg_s = load("gnwgs", gnwg_d, [64, 1], f32)
        gnbg_s = load("gnbgs", gnbg_d, [64, 1], f32)
        gnwr_s = load("gnwrs", gnwr_d, [128, 2], f32)
        gnbr_s = load("gnbrs", gnbr_d, [128, 2], f32)
        b2s_s = load("b2ss", b2s_d, [1, 1], f32)
        rss_s = load("rsss", rss_d, [1, 1], f32)

        # small persistent stat tiles
        xmp = pstat.tile([128, CW], f32, name="xmp", tag="xmp")
        edge_f = pstat.tile([128, 128], f32, name="edge_f", tag="edge_f")
        edge_b = pstat.tile([128, 128], bf16, name="edge_b", tag="edge_b")
        h1sum = pstat.tile([128, NT], f32, name="h1sum", tag="h1sum")
        h1sq = pstat.tile([128, NT], f32, name="h1sq", tag="h1sq")
        g1sum = pstat.tile([64, NT], f32, name="g1sum", tag="g1sum")
        g1sq = pstat.tile([64, NT], f32, name="g1sq", tag="g1sq")
        r1sum = [pstat.tile([128, NT], f32, name=f"r1sum{c}", tag=f"r1sum{c}") for c in range(2)]
        r1sq = [pstat.tile([128, NT], f32, name=f"r1sq{c}", tag=f"r1sq{c}") for c in range(2)]

        nc.vector.memset(xmp[:], 0.0)

        c_epsgn = pstat.tile([128, 1], f32, name="c_epsgn", tag="c_epsgn")
        nc.vector.memset(c_epsgn[:], EPS_GN)
        c_eps8 = pstat.tile([128, 1], f32, name="c_eps8", tag="c_eps8")
        nc.vector.memset(c_eps8[:], 1e-8)

        # ---------------- GN finalize helper ----------------
        def gn_finalize(nparts, chans_per_col, sum_tiles, sq_tiles, gmat, gnw, gnb, npix, prefix):
            k = len(sum_tiles)
            chan = pstat.tile([nparts, 2 * k], f32, name=f"{prefix}chan", tag=f"{prefix}chan")
            for j, t in enumerate(sum_tiles):
                nc.vector.tensor_reduce(chan[:, j:j + 1], t[:], axis=AX.X, op=ALU.add)
            for j, t in enumerate(sq_tiles):
                nc.vector.tensor_reduce(chan[:, k + j:k + j + 1], t[:], axis=AX.X, op=ALU.add)
            gp = ppsm.tile([nparts, 2 * k], f32, name=f"{prefix}gp", tag="sm")
            nc.tensor.matmul(gp[:], lhsT=gmat[:], rhs=chan[:], start=True, stop=True)
            mv = pstat.tile([nparts, k], f32, name=f"{prefix}mv", tag=f"{prefix}mv")
            ex2 = pstat.tile([nparts, k], f32, name=f"{prefix}ex2", tag=f"{prefix}ex2")
            nc.vector.tensor_scalar_mul(mv[:], gp[:, 0:k], 1.0 / npix)
            nc.vector.tensor_scalar_mul(ex2[:], gp[:, k:2 * k], 1.0 / npix)
            var = pstat.tile([nparts, k], f32, name=f"{prefix}var", tag=f"{prefix}var")
            nc.vector.tensor_tensor(out=var[:], in0=mv[:], in1=mv[:], op=ALU.mult)
            nc.vector.tensor_tensor(out=var[:], in0=ex2[:], in1=var[:], op=ALU.subtract)
            sd = pstat.tile([nparts, k], f32, name=f"{prefix}sd", tag=f"{prefix}sd")
            nc.scalar.activation(sd[:], var[:], AF.Sqrt, bias=c_epsgn[0:nparts, :], scale=1.0)
            rinv = pstat.tile([nparts, k], f32, name=f"{prefix}rinv", tag=f"{prefix}rinv")
            nc.vector.reciprocal(rinv[:], sd[:])
            av = pstat.tile([nparts, k], f32, name=f"{prefix}av", tag=f"{prefix}av")
            bv = pstat.tile([nparts, k], f32, name=f"{prefix}bv", tag=f"{prefix}bv")
            nc.vector.tensor_tensor(out=av[:], in0=gnw[:], in1=rinv[:], op=ALU.mult)
            nc.vector.scalar_tensor_tensor(out=bv[:], in0=mv[:], scalar=-1.0, in1=av[:],
                                           op0=ALU.mult, op1=ALU.mult)
            nc.vector.tensor_tensor(out=bv[:], in0=bv[:], in1=gnb[:], op=ALU.add)
            return av, bv

        # ================ P1: load fused, xm, laplacian, sobel ================
        pfz = tc.alloc_tile_pool(name="pfz", bufs=1, side="right")
        fz = [pfz.tile([128, LC], bf16, name=f"fz{c}", tag=f"fz{c}") for c in range(2)]
        nc.vector.memset(fz[0][:], 0.0)
        nc.vector.memset(fz[1][:], 0.0)
        # zero the DRAM hf spill canvases from the still-all-zero fz tiles
        nc.sync.dma_start(hfsp[0][:, :], fz[0][:])
        nc.sync.dma_start(hfsp[1][:, :], fz[1][:])

        for ti in range(NT):
            pxm = ppsm.tile([1, NPT], f32, name=f"pxm{ti}", tag="sm")
            for c in range(2):
                ft = pstage.tile([128, NPT], f32, name=f"p1f{ti}_{c}", tag="stf32", bufs=4)
                nc.sync.dma_start(ft[:], fused[c * 128:(c + 1) * 128, ti * NPT:(ti + 1) * NPT])
                bt = pstage.tile([128, NPT], bf16, name=f"p1b{ti}_{c}", tag="stbf", bufs=4)
                nc.vector.tensor_copy(bt[:], ft[:])
                nc.sync.dma_start(win(fz[c], ti, 0), bt[:])
                nc.sync.dma_start(fzlin[c][:, ti * NPT:(ti + 1) * NPT], bt[:])
                nc.tensor.matmul(pxm[:], lhsT=ones1_s[:], rhs=ft[:],
                                 start=(c == 0), stop=(c == 1))
            s1 = pstage.tile([1, NPT], f32, name=f"p1s{ti}", tag="st1", bufs=2)
            nc.scalar.activation(s1[:], pxm[:], AF.Copy)
            nc.sync.dma_start(xmp[ti * RT:(ti + 1) * RT, 1:1 + W], s1[:])

        # laplacian via scaled-identity matmuls -> hf spill canvases in DRAM
        for c in range(2):
            for ti in range(NT):
                pt = pps.tile([128, NPT], f32, name=f"lap{c}_{ti}", tag="acc")
                for i, ((ty, tx), coef) in enumerate(LAP_TAPS):
                    lw = idlap_s[:, 128:256] if coef == -4.0 else idlap_s[:, 0:128]
                    nc.tensor.matmul(pt[:], lhsT=lw, rhs=win(fz[c], ti, ty * CW + tx),
                                     start=(i == 0), stop=(i == len(LAP_TAPS) - 1))
                hb = pstage.tile([128, NPT], bf16, name=f"lapb{c}_{ti}", tag="stbf", bufs=4)
                nc.scalar.activation(hb[:], pt[:], AF.Copy)
                base = (ti * RT + 1) * CW + 1
                dst = hfsp[c][:, base:base + RT * CW].rearrange("p (r x) -> p r x", r=RT)[:, :, 0:W]
                nc.sync.dma_start(dst, hb[:])

        # --- sobel / edge map (all fp32) ---
        gxp = pstat.tile([128, 128], f32, name="gxp", tag="gxp")
        gyp = pstat.tile([128, 128], f32, name="gyp", tag="gyp")
        nc.vector.tensor_tensor(out=gxp[:], in0=xmp[:, 2:130], in1=xmp[:, 0:128], op=ALU.subtract)
        nc.vector.scalar_tensor_tensor(out=gyp[:], in0=xmp[:, 1:129], scalar=2.0,
                                       in1=xmp[:, 0:128], op0=ALU.mult, op1=ALU.add)
        nc.vector.tensor_tensor(out=gyp[:], in0=gyp[:], in1=xmp[:, 2:130], op=ALU.add)
        psx = ppsm.tile([128, 128], f32, name="psx", tag="sm")
        nc.tensor.matmul(psx[:], lhsT=bsbd_s[:, 0:128], rhs=gxp[:], start=True, stop=True)
        psy = ppsm.tile([128, 128], f32, name="psy", tag="sm")
        nc.tensor.matmul(psy[:], lhsT=bsbd_s[:, 128:256], rhs=gyp[:], start=True, stop=True)
        mag = pstat.tile([128, 128], f32, name="mag", tag="mag")
        m2 = pstat.tile([128, 128], f32, name="m2t", tag="m2t")
        nc.scalar.activation(mag[:], psx[:], AF.Square)
        nc.scalar.activation(m2[:], psy[:], AF.Square)
        nc.vector.tensor_tensor(out=mag[:], in0=mag[:], in1=m2[:], op=ALU.add)
        nc.scalar.activation(mag[:], mag[:], AF.Sqrt, bias=c_eps8[:], scale=1.0)
        mx = pstat.tile([128, 1], f32, name="mx", tag="mx")
        mn = pstat.tile([128, 1], f32, name="mn", tag="mn")
        nc.vector.tensor_reduce(mx[:], mag[:], axis=AX.X, op=ALU.max)
        nc.vector.tensor_reduce(mn[:], mag[:], axis=AX.X, op=ALU.min)
        mxs = pstat.tile([1, 1], f32, name="mxs", tag="mxs")
        nmn = pstat.tile([1, 1], f32, name="nmn", tag="nmn")
        mneg = pstat.tile([128, 1], f32, name="mneg", tag="mneg")
        nc.vector.tensor_scalar_mul(mneg[:], mn[:], -1.0)
        nc.gpsimd.tensor_reduce(mxs[:], mx[:], axis=AX.C, op=ALU.max)
        nc.gpsimd.tensor_reduce(nmn[:], mneg[:], axis=AX.C, op=ALU.max)  # = -emin
        den = pstat.tile([1, 1], f32, name="den", tag="den")
        nc.vector.tensor_tensor(out=den[:], in0=mxs[:], in1=nmn[:], op=ALU.add)
        nc.vector.tensor_scalar_add(den[:], den[:], 1e-8)
        rec = pstat.tile([1, 1], f32, name="rec", tag="rec")
        nc.vector.reciprocal(rec[:], den[:])
        be = pstat.tile([1, 1], f32, name="be", tag="be")
        nc.vector.tensor_tensor(out=be[:], in0=nmn[:], in1=rec[:], op=ALU.mult)
        reca = pstat.tile([128, 1], f32, name="reca", tag="reca")
        bea = pstat.tile([128, 1], f32, name="bea", tag="bea")
        nc.gpsimd.partition_broadcast(reca[:], rec[:])
        nc.gpsimd.partition_broadcast(bea[:], be[:])
        nc.scalar.activation(edge_f[:], mag[:], AF.Identity, scale=reca[:], bias=bea[:])
        nc.sync.dma_start(edge_o[:, :], edge_f[:])
        nc.vector.tensor_copy(edge_b[:], edge_f[:])
        nc.sync.dma_start(eflat_sp[0:1, :], edge_b[:])

        # ================ P2a: gate conv (uses fz + edge9) ================
        pg1 = tc.alloc_tile_pool(name="pg1", bufs=1)
        g1n = pg1.tile([64, LC], bf16, name="g1n", tag="g1n")
        nc.vector.memset(g1n[:], 0.0)

        pe9 = tc.alloc_tile_pool(name="pe9", bufs=1, side="right")
        e9 = pe9.tile([9, LC], bf16, name="e9", tag="e9")
        nc.gpsimd.memset(e9[:], 0.0)
        for t, (ty, tx) in enumerate(TAPS):
            base_t = (1 - ty) * CW + (1 - tx)
            dst = e9[t:t + 1, base_t:base_t + H * CW].rearrange("p (r x) -> p r x", r=H)[:, :, 0:W]
            nc.sync.dma_start(dst, edge_b[:])

        scr64 = pstat.tile([64, NPT], f32, name="scr64", tag="scr64")
        for ti in range(NT):
            pt = pps.tile([64, NPT], f32, name=f"g1p{ti}", tag="acc")
            for c in range(2):
                for t in range(9):
                    col0 = (t * 2 + c) * 64
                    nc.tensor.matmul(pt[:], lhsT=wg1_s[:, col0:col0 + 64],
                                     rhs=win(fz[c], ti, TAPS[t][0] * CW + TAPS[t][1]),
                                     start=(c == 0 and t == 0), stop=False)
            nc.tensor.matmul(pt[:], lhsT=wg1e_s[:], rhs=win(e9, ti, 0, parts=9),
                             start=False, stop=True)
            nc.scalar.activation(win(g1n, ti, 0), pt[:], AF.Copy,
                                 accum_out=g1sum[:, ti:ti + 1])
            nc.scalar.activation(scr64[:], pt[:], AF.Square,
                                 accum_out=g1sq[:, ti:ti + 1])
        pe9.release()

        ag, bg = gn_finalize(64, 8, [g1sum], [g1sq], gmat8_s, gnwg_s, gnbg_s,
                             8 * HW, "g")
        nc.scalar.activation(interior(g1n), interior(g1n), AF.Gelu,
                             scale=ag[:, 0:1], bias=bg[:, 0:1])

        # gate2 1x1 conv + sigmoid -> gate_o + scaled bf16 gate spill
        pgsb = tc.alloc_tile_pool(name="pgsb", bufs=1)
        gsb = pgsb.tile([1, HW], bf16, name="gsb", tag="gsb")
        for ti in range(NT):
            pt1 = ppsm.tile([1, NPT], f32, name=f"g2p{ti}", tag="sm")
            nc.tensor.matmul(pt1[:], lhsT=wg2_s[:], rhs=win(g1n, ti, 0),
                             start=True, stop=True)
            s1 = pstage.tile([1, NPT], f32, name=f"g2s{ti}", tag="st1", bufs=2)
            nc.scalar.activation(s1[:], pt1[:], AF.Sigmoid, bias=b2s_s[:], scale=1.0)
            nc.sync.dma_start(gate_o[0:1, ti * NPT:(ti + 1) * NPT], s1[:])
            nc.vector.tensor_scalar(out=gsb[:, ti * NPT:(ti + 1) * NPT], in0=s1[:],
                                    scalar1=rss_s[:], scalar2=None, op0=ALU.mult)
        nc.sync.dma_start(gspill[0:1, :], gsb[:])
        pgsb.release()
        pg1.release()
        pfz.release()

        # ================ P2b: hf1 conv (reload hf canvases) ================
        phf = tc.alloc_tile_pool(name="phf", bufs=1, side="right")
        hfc = [phf.tile([128, LC], bf16, name=f"hfc{c}", tag=f"hfc{c}") for c in range(2)]
        nc.sync.dma_start(hfc[0][:], hfsp[0][:, :])
        nc.sync.dma_start(hfc[1][:], hfsp[1][:, :])
        ph1 = tc.alloc_tile_pool(name="ph1", bufs=1)
        h1n = ph1.tile([128, LC], bf16, name="h1n", tag="h1n")
        nc.vector.memset(h1n[:], 0.0)

        scr128 = pstat.tile([128, NPT], f32, name="scr128", tag="scr128")
        for ti in range(NT):
            pt = pps.tile([128, NPT], f32, name=f"h1p{ti}", tag="acc")
            k = 0
            for c in range(2):
                for t in range(9):
                    col0 = (t * 2 + c) * 128
                    nc.tensor.matmul(pt[:], lhsT=whf1_s[:, col0:col0 + 128],
                                     rhs=win(hfc[c], ti, TAPS[t][0] * CW + TAPS[t][1]),
                                     start=(k == 0), stop=(k == 17))
                    k += 1
            nc.scalar.activation(win(h1n, ti, 0), pt[:], AF.Copy,
                                 accum_out=h1sum[:, ti:ti + 1])
            nc.scalar.activation(scr128[:], pt[:], AF.Square,
                                 accum_out=h1sq[:, ti:ti + 1])
        phf.release()

        ah, bh = gn_finalize(128, 16, [h1sum], [h1sq], gmat16_s, gnwh_s, gnbh_s,
                             16 * HW, "h")
        nc.scalar.activation(interior(h1n), interior(h1n), AF.Gelu,
                             scale=ah[:, 0:1], bias=bh[:, 0:1])

        # ================ P3: hf2 conv -> rin = fused + edge*hf_feat ================
        prin = tc.alloc_tile_pool(name="prin", bufs=1, side="right")
        rin = [prin.tile([128, LC], bf16, name=f"rin{c}", tag=f"rin{c}") for c in range(2)]
        pedge = tc.alloc_tile_pool(name="pedge", bufs=1, side="right")
        ebc = pedge.tile([128, HW], bf16, name="ebc", tag="ebc")
        pesb = tc.alloc_tile_pool(name="pesb", bufs=2, side="right")
        QB = 2048
        for q in range(HW // QB):
            esb = pesb.tile([1, QB], bf16, name=f"esb{q}", tag="esb")
            nc.sync.dma_start(esb[:], eflat_sp[0:1, q * QB:(q + 1) * QB])
            nc.gpsimd.partition_broadcast(ebc[:, q * QB:(q + 1) * QB], esb[:])
        pesb.release()
        nc.vector.memset(rin[0][:], 0.0)
        nc.vector.memset(rin[1][:], 0.0)

        for ti in range(NT):
            for m in range(2):
                pt = pps.tile([128, NPT], f32, name=f"h2p{ti}_{m}", tag="acc")
                for t in range(9):
                    col0 = (t * 2 + m) * 128
                    nc.tensor.matmul(pt[:], lhsT=whf2_s[:, col0:col0 + 128],
                                     rhs=win(h1n, ti, TAPS[t][0] * CW + TAPS[t][1]),
                                     start=(t == 0), stop=(t == 8))
                t1 = pstage.tile([128, NPT], f32, name=f"h2t{ti}_{m}", tag="stf32b", bufs=2)
                nc.vector.tensor_tensor(out=t1[:], in0=pt[:],
                                        in1=ebc[:, ti * NPT:(ti + 1) * NPT], op=ALU.mult)
                fb = pstage.tile([128, NPT], bf16, name=f"h2f{ti}_{m}", tag="stbf", bufs=4)
                nc.sync.dma_start(fb[:], fzlin[m][:, ti * NPT:(ti + 1) * NPT])
                nc.vector.tensor_tensor(out=win(rin[m], ti, 0), in0=t1[:], in1=fb[:], op=ALU.add)
        ph1.release()
        pedge.release()

        # ================ P4: ref1 conv ================
        pr1 = tc.alloc_tile_pool(name="pr1", bufs=1)
        r1n = [pr1.tile([128, LC], bf16, name=f"r1n{c}", tag=f"r1n{c}") for c in range(2)]
        nc.vector.memset(r1n[0][:], 0.0)
        nc.vector.memset(r1n[1][:], 0.0)

        for ti in range(NT):
            for m in range(2):
                pt = pps.tile([128, NPT], f32, name=f"r1p{ti}_{m}", tag="acc")
                k = 0
                for c in range(2):
                    for t in range(9):
                        col0 = ((t * 2 + c) * 2 + m) * 128
                        nc.tensor.matmul(pt[:], lhsT=wr1_s[:, col0:col0 + 128],
                                         rhs=win(rin[c], ti, TAPS[t][0] * CW + TAPS[t][1]),
                                         start=(k == 0), stop=(k == 17))
                        k += 1
                nc.scalar.activation(win(r1n[m], ti, 0), pt[:], AF.Copy,
                                     accum_out=r1sum[m][:, ti:ti + 1])
                nc.scalar.activation(scr128[:], pt[:], AF.Square,
                                     accum_out=r1sq[m][:, ti:ti + 1])
        prin.release()

        ar, br = gn_finalize(128, 32, r1sum, r1sq, gmat32_s, gnwr_s, gnbr_s,
                             32 * HW, "r")
        for m in range(2):
            nc.scalar.activation(interior(r1n[m]), interior(r1n[m]), AF.Gelu,
                                 scale=ar[:, m:m + 1], bias=br[:, m:m + 1])

        # ================ P5: ref2 conv -> refined ================
        pgb = tc.alloc_tile_pool(name="pgb", bufs=1)
        gbc = pgb.tile([128, HW], bf16, name="gbc", tag="gbc")
        pgsb2 = tc.alloc_tile_pool(name="pgsb2", bufs=2)
        QB = 2048
        for q in range(HW // QB):
            gsb2 = pgsb2.tile([1, QB], bf16, name=f"gsb2{q}", tag="gsb2")
            nc.sync.dma_start(gsb2[:], gspill[0:1, q * QB:(q + 1) * QB])
            nc.gpsimd.partition_broadcast(gbc[:, q * QB:(q + 1) * QB], gsb2[:])
        pgsb2.release()

        for ti in range(NT):
            for m in range(2):
                pt = pps.tile([128, NPT], f32, name=f"r2p{ti}_{m}", tag="acc")
                k = 0
                for c in range(2):
                    for t in range(9):
                        col0 = ((t * 2 + c) * 2 + m) * 128
                        nc.tensor.matmul(pt[:], lhsT=wr2_s[:, col0:col0 + 128],
                                         rhs=win(r1n[c], ti, TAPS[t][0] * CW + TAPS[t][1]),
                                         start=(k == 0), stop=(k == 17))
                        k += 1
                t1 = pstage.tile([128, NPT], f32, name=f"r2t{ti}_{m}", tag="stf32b", bufs=2)
                nc.vector.tensor_tensor(out=t1[:], in0=pt[:],
                                        in1=gbc[:, ti * NPT:(ti + 1) * NPT], op=ALU.mult)
                ff = pstage.tile([128, NPT], f32, name=f"r2f{ti}_{m}", tag="stf32", bufs=4)
                nc.sync.dma_start(ff[:], fused[m * 128:(m + 1) * 128, ti * NPT:(ti + 1) * NPT])
                ot = pstage.tile([128, NPT], f32, name=f"r2o{ti}_{m}", tag="stf32c", bufs=2)
                nc.vector.tensor_tensor(out=ot[:], in0=t1[:], in1=ff[:], op=ALU.add)
                nc.sync.dma_start(refined[m * 128:(m + 1) * 128, ti * NPT:(ti + 1) * NPT], ot[:])
        pgb.release()
        pr1.release()

    nc.compile()
    _CACHE["built"] = nc
    return nc


# ---------------- host orchestration ----------------
def _prep_inputs(inputs):
    f32 = np.float32
    consts = _host_consts()
    shared = {
        "whf1": _pack_conv(np.asarray(inputs["hf_w1"], f32), 2, 1, 128),
        "whf2": _pack_conv(np.asarray(inputs["hf_w2"], f32), 1, 2, 128),
        "wg1": _pack_conv(np.asarray(inputs["gate_w1"], f32)[:, :256], 2, 1, 64),
        "wg1e": np.ascontiguousarray(
            np.asarray(inputs["gate_w1"], f32)[:, 256, :, :].reshape(64, 9).T
        ).astype(ml_dtypes.bfloat16),
        "wg2": np.asarray(inputs["gate_w2"], f32).reshape(1, 64).T.astype(ml_dtypes.bfloat16).copy(),
        "wr1": _pack_conv(np.asarray(inputs["ref_w1"], f32), 2, 2, 128),
        "wr2": _pack_conv(np.asarray(inputs["ref_w2"], f32), 2, 2, 128),
        "bsbd": consts["bsbd"],
        "gmat16": consts["gmat16"], "gmat32": consts["gmat32"], "gmat8": consts["gmat8"],
        "idlap": consts["idlap"], "ones1": consts["ones1"],
        "gnwh": np.asarray(inputs["hf_gn_w"], f32).reshape(128, 1),
        "gnbh": np.asarray(inputs["hf_gn_b"], f32).reshape(128, 1),
        "gnwg": np.asarray(inputs["gate_gn_w"], f32).reshape(64, 1),
        "gnbg": np.asarray(inputs["gate_gn_b"], f32).reshape(64, 1),
        "gnwr": np.asarray(inputs["ref_gn_w"], f32).reshape(2, 128).T.copy(),
        "gnbr": np.asarray(inputs["ref_gn_b"], f32).reshape(2, 128).T.copy(),
        "b2s": np.asarray(inputs["gate_b2"], f32).reshape(1, 1),
        "rss": np.asarray(inputs["residual_scale"], f32).reshape(1, 1),
    }
    fused = np.asarray(inputs["fused"], f32)
    per_core = []
    for b in range(8):
        m = dict(shared)
        m["fused"] = np.ascontiguousarray(fused[b].reshape(256, HW))
        per_core.append(m)
    return per_core


def _postprocess(results):
    refined = np.stack([r["refined"].reshape(256, H, W) for r in results]).astype(np.float32)
    edge = np.stack([r["edge_o"].reshape(1, H, W) for r in results]).astype(np.float32)
    gate = np.stack([r["gate_o"].reshape(-1) for r in results])
    gate_mean = np.float32(np.mean(gate.astype(np.float64)))
    return refined, edge, gate_mean


def kernel(**inputs):
    nc = _build()
    per_core = _prep_inputs(inputs)
    from concourse.bass_utils import run_bass_kernel_spmd
    res = run_bass_kernel_spmd(nc, per_core, list(range(8)))
    return _postprocess(res.results)


# ---------------- single-core simulator check (used by test.py) ----------------
def sim_one_core(inputs, core=0):
    import concourse.bass_interp as bass_interp
    nc = _build()
    per_core = _prep_inputs(inputs)
    sim = bass_interp.CoreSim(nc)
    for k, v in per_core[core].items():
        sim.tensor(k)[:] = v
    sim.simulate()
    out = {k: np.array(sim.tensor(k)) for k in ("refined", "edge_o", "gate_o")}
    return out


# revision 25
# speedup vs baseline: 1.0588x; 1.0588x over previous
"""Trainium2 Bass kernel for BoundaryRefinementModule.

One image per NeuronCore (B=8 over 8 cores, pure data parallel).
Convs are 9-shifted-matmuls accumulating in PSUM; activations in bf16,
accumulation + stats + pointwise in fp32.

Layout: activations live in SBUF "canvases" of shape [128, 16904] bf16,
where pixel (h, w) of a 128x128 image sits at free index (h+1)*130+(w+1)
(a 130x130 zero-padded canvas, flattened; +4 tail pad so conv windows
never slice out of range).
"""

import numpy as np
import ml_dtypes

# ---------------- geometry ----------------
H = W = 128
HW = H * W
CW = 130                  # canvas row width
LC = 16904                # canvas free elems (130*130 + 4 tail)
ORG = CW + 1              # canvas index of pixel (0,0)
RT = 4                    # image rows per spatial tile
NT = H // RT              # 32 spatial tiles
NPT = RT * W              # 512 pixels per tile
TAPS = [(ty, tx) for ty in (-1, 0, 1) for tx in (-1, 0, 1)]
LAP_TAPS = [((-1, 0), 1.0), ((0, -1), 1.0), ((0, 0), -4.0), ((0, 1), 1.0), ((1, 0), 1.0)]
EPS_GN = 1e-5

_CACHE = {}


# ---------------- host-side weight packing ----------------
def _pack_conv(w, cin_chunks, cout_chunks, mc):
    """w: (Cout, Cin, 3, 3) fp32 -> (128, 9*cin_chunks*cout_chunks*mc) bf16.

    Column block for (tap t, cin chunk c, cout chunk m):
      col0 = ((t*cin_chunks + c)*cout_chunks + m)*mc
      A[k, col0+j] = w[m*mc+j, c*128+k, ty+1, tx+1]
    """
    ncol = 9 * cin_chunks * cout_chunks * mc
    A = np.zeros((128, ncol), np.float32)
    for t, (ty, tx) in enumerate(TAPS):
        for c in range(cin_chunks):
            for m in range(cout_chunks):
                col0 = ((t * cin_chunks + c) * cout_chunks + m) * mc
                blk = w[m * mc:(m + 1) * mc, c * 128:(c + 1) * 128, ty + 1, tx + 1]
                A[:, col0:col0 + mc] = blk.T
    return A.astype(ml_dtypes.bfloat16)


def _host_consts():
    # Vertical Sobel band matrices (SAME zero boundary), fp32.
    Bs = np.zeros((128, 128), np.float32)   # smoothing [1,2,1]
    Bd = np.zeros((128, 128), np.float32)   # difference [-1,0,1]
    for h in range(128):
        for d in (-1, 0, 1):
            k = h + d
            if 0 <= k < 128:
                Bs[k, h] = (1.0, 2.0, 1.0)[d + 1]
                Bd[k, h] = float(d)
    bsbd = np.concatenate([Bs, Bd], axis=1)  # (128, 256)

    def block_ind(p, g):
        M = np.zeros((p, p), np.float32)
        for i in range(p):
            M[i, (i // g) * g:(i // g + 1) * g] = 1.0
        return M

    gmat16 = block_ind(128, 16)   # hf GN: 128 ch, groups of 16
    gmat32 = block_ind(128, 32)   # ref GN: per-chunk groups of 32
    gmat8 = block_ind(64, 8)      # gate GN: 64 ch, groups of 8

    idlap = np.zeros((128, 256), np.float32)
    idlap[:, 0:128] = np.eye(128)
    idlap[:, 128:256] = -4.0 * np.eye(128)
    ones1 = np.full((128, 1), 1.0 / 256.0, np.float32)  # channel-mean weights
    return {
        "bsbd": bsbd.astype(np.float32),
        "gmat16": gmat16, "gmat32": gmat32, "gmat8": gmat8,
        "idlap": idlap.astype(ml_dtypes.bfloat16),
        "ones1": ones1.astype(np.float32),
    }


# ---------------- bass program ----------------
def _build():
    if "built" in _CACHE:
        return _CACHE["built"]
    from contextlib import ExitStack
    import concourse.bacc as bacc
    import concourse.tile as tile
    from concourse import mybir
    import concourse.bass as bass

    f32 = mybir.dt.float32
    bf16 = mybir.dt.bfloat16
    AF = mybir.ActivationFunctionType
    ALU = mybir.AluOpType
    AX = mybir.AxisListType

    nc = bacc.Bacc("TRN2", target_bir_lowering=False, debug=False, num_devices=8)

    # -------- DRAM I/O --------
    fused = nc.dram_tensor("fused", [256, HW], f32, kind="ExternalInput").ap()
    whf1 = nc.dram_tensor("whf1", [128, 2304], bf16, kind="ExternalInput").ap()
    whf2 = nc.dram_tensor("whf2", [128, 2304], bf16, kind="ExternalInput").ap()
    wg1 = nc.dram_tensor("wg1", [128, 1152], bf16, kind="ExternalInput").ap()
    wg1e = nc.dram_tensor("wg1e", [9, 64], bf16, kind="ExternalInput").ap()
    wg2 = nc.dram_tensor("wg2", [64, 1], bf16, kind="ExternalInput").ap()
    wr1 = nc.dram_tensor("wr1", [128, 4608], bf16, kind="ExternalInput").ap()
    wr2 = nc.dram_tensor("wr2", [128, 4608], bf16, kind="ExternalInput").ap()
    bsbd_d = nc.dram_tensor("bsbd", [128, 256], f32, kind="ExternalInput").ap()
    gmat16_d = nc.dram_tensor("gmat16", [128, 128], f32, kind="ExternalInput").ap()
    gmat32_d = nc.dram_tensor("gmat32", [128, 128], f32, kind="ExternalInput").ap()
    gmat8_d = nc.dram_tensor("gmat8", [64, 64], f32, kind="ExternalInput").ap()
    idlap_d = nc.dram_tensor("idlap", [128, 256], bf16, kind="ExternalInput").ap()
    ones1_d = nc.dram_tensor("ones1", [128, 1], f32, kind="ExternalInput").ap()
    gnwh_d = nc.dram_tensor("gnwh", [128, 1], f32, kind="ExternalInput").ap()
    gnbh_d = nc.dram_tensor("gnbh", [128, 1], f32, kind="ExternalInput").ap()
    gnwg_d = nc.dram_tensor("gnwg", [64, 1], f32, kind="ExternalInput").ap()
    gnbg_d = nc.dram_tensor("gnbg", [64, 1], f32, kind="ExternalInput").ap()
    gnwr_d = nc.dram_tensor("gnwr", [128, 2], f32, kind="ExternalInput").ap()
    gnbr_d = nc.dram_tensor("gnbr", [128, 2], f32, kind="ExternalInput").ap()
    b2s_d = nc.dram_tensor("b2s", [1, 1], f32, kind="ExternalInput").ap()
    rss_d = nc.dram_tensor("rss", [1, 1], f32, kind="ExternalInput").ap()

    refined = nc.dram_tensor("refined", [256, HW], f32, kind="ExternalOutput").ap()
    edge_o = nc.dram_tensor("edge_o", [128, 128], f32, kind="ExternalOutput").ap()
    gate_o = nc.dram_tensor("gate_o", [1, HW], f32, kind="ExternalOutput").ap()



    def win(cv, ti, off, parts=None):
        """Conv rhs / write window: (P, RT, 128), canvas row stride CW."""
        base = (ti * RT + 1) * CW + 1 + off
        w = cv[:, base:base + RT * CW].rearrange("p (r x) -> p r x", r=RT)
        return w[:, :, 0:W]

    def interior(cv):
        w = cv[:, CW:CW + H * CW].rearrange("p (r x) -> p r x", r=H)
        return w[:, :, 1:1 + W]

    def zero_pads(nc, cv):
        # top row + col0 of row 1; the col129/col0 pair between rows; bottom row + tail
        nc.vector.memset(cv[:, 0:CW + 1], 0.0)
        mid = cv[:, CW + W + 1:CW + W + 1 + 127 * CW].rearrange(
            "p (r x) -> p r x", r=127)[:, :, 0:2]
        nc.vector.memset(mid, 0.0)
        nc.vector.memset(cv[:, LC - CW - 5:LC], 0.0)

    with tile.TileContext(nc) as tc, ExitStack() as CTX:
        # ---------------- persistent pools ----------------
        pw = CTX.enter_context(tc.tile_pool(name="pw", bufs=1))
        pstat = CTX.enter_context(tc.tile_pool(name="pstat", bufs=1))
        pstage = CTX.enter_context(tc.tile_pool(name="pstage", bufs=1))
        pps = CTX.enter_context(tc.tile_pool(name="pps", bufs=6, space="PSUM"))
        ppsm = CTX.enter_context(tc.tile_pool(name="ppsm", bufs=2, space="PSUM"))
        pdram = CTX.enter_context(tc.tile_pool(name="pdram", bufs=1, space="DRAM"))

        # DRAM spill tiles (tracked by Tile for DMA ordering)
        hfsp = [pdram.tile([128, LC], bf16, name=f"hfsp{c}", tag=f"hfsp{c}") for c in range(2)]
        fzlin = [pdram.tile([128, HW], bf16, name=f"fzlin{c}", tag=f"fzlin{c}") for c in range(2)]
        eflat_sp = pdram.tile([1, HW], bf16, name="eflat_sp", tag="eflat_sp")
        gspill = pdram.tile([1, HW], bf16, name="gspill", tag="gspill")

        # weights / consts to SBUF
        def load(name, src, shape, dt):
            t = pw.tile(shape, dt, name=name, tag=name)
            nc.sync.dma_start(t[:], src)
            return t

        whf1_s = load("whf1s", whf1, [128, 2304], bf16)
        whf2_s = load("whf2s", whf2, [128, 2304], bf16)
        wg1_s = load("wg1s", wg1, [128, 1152], bf16)
        wg1e_s = load("wg1es", wg1e, [9, 64], bf16)
        wg2_s = load("wg2s", wg2, [64, 1], bf16)
        wr1_s = load("wr1s", wr1, [128, 4608], bf16)
        wr2_s = load("wr2s", wr2, [128, 4608], bf16)
        bsbd_s = load("bsbds", bsbd_d, [128, 256], f32)
        gmat16_s = load("gmat16s", gmat16_d, [128, 128], f32)
        gmat32_s = load("gmat32s", gmat32_d, [128, 128], f32)
        gmat8_s = load("gmat8s", gmat8_d, [64, 64], f32)
        idlap_s = load("idlaps", idlap_d, [128, 256], bf16)
        ones1_s = load("ones1s", ones1_d, [128, 1], f32)
        gnwh_s = load("gnwhs", gnwh_d, [128, 1], f32)
        gnbh_s = load("gnbhs", gnbh_d, [128, 1], f32)
        gnwg_s = load("gnwgs", gnwg_d, [64, 1], f32)
        gnbg_s = load("gnbgs", gnbg_d, [64, 1], f32)
        gnwr_s = load("gnwrs", gnwr_d, [128, 2], f32)
        gnbr_s = load("gnbrs", gnbr_d, [128, 2], f32)
        b2s_s = load("b2ss", b2s_d, [1, 1], f32)
        rss_s = load("rsss", rss_d, [1, 1], f32)

        # small persistent stat tiles
        xmp = pstat.tile([128, CW], f32, name="xmp", tag="xmp")
        edge_f = pstat.tile([128, 128], f32, name="edge_f", tag="edge_f")
        edge_b = pstat.tile([128, 128], bf16, name="edge_b", tag="edge_b")
        h1sum = pstat.tile([128, NT], f32, name="h1sum", tag="h1sum")
        h1sq = pstat.tile([128, NT], f32, name="h1sq", tag="h1sq")
        g1sum = pstat.tile([64, NT], f32, name="g1sum", tag="g1sum")
        g1sq = pstat.tile([64, NT], f32, name="g1sq", tag="g1sq")
        r1sum = [pstat.tile([128, NT], f32, name=f"r1sum{c}", tag=f"r1sum{c}") for c in range(2)]
        r1sq = [pstat.tile([128, NT], f32, name=f"r1sq{c}", tag=f"r1sq{c}") for c in range(2)]

        nc.vector.memset(xmp[:], 0.0)

        c_epsgn = pstat.tile([128, 1], f32, name="c_epsgn", tag="c_epsgn")
        nc.vector.memset(c_epsgn[:], EPS_GN)
        c_eps8 = pstat.tile([128, 1], f32, name="c_eps8", tag="c_eps8")
        nc.vector.memset(c_eps8[:], 1e-8)

        # ---------------- GN finalize helper ----------------
        def gn_finalize(nparts, chans_per_col, sum_tiles, sq_tiles, gmat, gnw, gnb, npix, prefix):
            k = len(sum_tiles)
            chan = pstat.tile([nparts, 2 * k], f32, name=f"{prefix}chan", tag=f"{prefix}chan")
            for j, t in enumerate(sum_tiles):
                nc.vector.tensor_reduce(chan[:, j:j + 1], t[:], axis=AX.X, op=ALU.add)
            for j, t in enumerate(sq_tiles):
                nc.vector.tensor_reduce(chan[:, k + j:k + j + 1], t[:], axis=AX.X, op=ALU.add)
            gp = ppsm.tile([nparts, 2 * k], f32, name=f"{prefix}gp", tag="sm")
            nc.tensor.matmul(gp[:], lhsT=gmat[:], rhs=chan[:], start=True, stop=True)
            mv = pstat.tile([nparts, k], f32, name=f"{prefix}mv", tag=f"{prefix}mv")
            ex2 = pstat.tile([nparts, k], f32, name=f"{prefix}ex2", tag=f"{prefix}ex2")
            nc.vector.tensor_scalar_mul(mv[:], gp[:, 0:k], 1.0 / npix)
            nc.vector.tensor_scalar_mul(ex2[:], gp[:, k:2 * k], 1.0 / npix)
            var = pstat.tile([nparts, k], f32, name=f"{prefix}var", tag=f"{prefix}var")
            nc.vector.tensor_tensor(out=var[:], in0=mv[:], in1=mv[:], op=ALU.mult)
            nc.vector.tensor_tensor(out=var[:], in0=ex2[:], in1=var[:], op=ALU.subtract)
            sd = pstat.tile([nparts, k], f32, name=f"{prefix}sd", tag=f"{prefix}sd")
            nc.scalar.activation(sd[:], var[:], AF.Sqrt, bias=c_epsgn[0:nparts, :], scale=1.0)
            rinv = pstat.tile([nparts, k], f32, name=f"{prefix}rinv", tag=f"{prefix}rinv")
            nc.vector.reciprocal(rinv[:], sd[:])
            av = pstat.tile([nparts, k], f32, name=f"{prefix}av", tag=f"{prefix}av")
            bv = pstat.tile([nparts, k], f32, name=f"{prefix}bv", tag=f"{prefix}bv")
            nc.vector.tensor_tensor(out=av[:], in0=gnw[:], in1=rinv[:], op=ALU.mult)
            nc.vector.scalar_tensor_tensor(out=bv[:], in0=mv[:], scalar=-1.0, in1=av[:],
                                           op0=ALU.mult, op1=ALU.mult)
            nc.vector.tensor_tensor(out=bv[:], in0=bv[:], in1=gnb[:], op=ALU.add)
            return av, bv

        # ================ P1: load fused, xm, laplacian, sobel ================
        pfz = tc.alloc_tile_pool(name="pfz", bufs=1, side="right")
        fz = [pfz.tile([128, LC], bf16, name=f"fz{c}", tag=f"fz{c}") for c in range(2)]
        # full memset here: fz also seeds the zero pads of the hf spill canvases
        nc.vector.memset(fz[0][:], 0.0)
        nc.vector.memset(fz[1][:], 0.0)
        nc.sync.dma_start(hfsp[0][:, :], fz[0][:])
        nc.sync.dma_start(hfsp[1][:, :], fz[1][:])

        def lap_tile(c, ti):
            pt = pps.tile([128, NPT], f32, name=f"lap{c}_{ti}", tag="acc")
            for i, ((ty, tx), coef) in enumerate(LAP_TAPS):
                lw = idlap_s[:, 128:256] if coef == -4.0 else idlap_s[:, 0:128]
                nc.tensor.matmul(pt[:], lhsT=lw, rhs=win(fz[c], ti, ty * CW + tx),
                                 start=(i == 0), stop=(i == len(LAP_TAPS) - 1))
            hb = pstage.tile([128, NPT], bf16, name=f"lapb{c}_{ti}", tag="stbf", bufs=6)
            nc.scalar.activation(hb[:], pt[:], AF.Copy)
            base = (ti * RT + 1) * CW + 1
            dst = hfsp[c][:, base:base + RT * CW].rearrange("p (r x) -> p r x", r=RT)[:, :, 0:W]
            nc.sync.dma_start(dst, hb[:])

        for ti in range(NT):
            pxm = ppsm.tile([1, NPT], f32, name=f"pxm{ti}", tag="sm")
            for c in range(2):
                ft = pstage.tile([128, NPT], f32, name=f"p1f{ti}_{c}", tag="stf32", bufs=6)
                nc.sync.dma_start(ft[:], fused[c * 128:(c + 1) * 128, ti * NPT:(ti + 1) * NPT])
                bt = pstage.tile([128, NPT], bf16, name=f"p1b{ti}_{c}", tag="stbf", bufs=6)
                nc.vector.tensor_copy(bt[:], ft[:])
                nc.sync.dma_start(win(fz[c], ti, 0), bt[:])
                nc.sync.dma_start(fzlin[c][:, ti * NPT:(ti + 1) * NPT], bt[:])
                nc.tensor.matmul(pxm[:], lhsT=ones1_s[:], rhs=ft[:],
                                 start=(c == 0), stop=(c == 1))
            s1 = pstage.tile([1, NPT], f32, name=f"p1s{ti}", tag="st1", bufs=2)
            nc.scalar.activation(s1[:], pxm[:], AF.Copy)
            nc.sync.dma_start(xmp[ti * RT:(ti + 1) * RT, 1:1 + W], s1[:])
            if ti >= 1:
                for c in range(2):
                    lap_tile(c, ti - 1)
        for c in range(2):
            lap_tile(c, NT - 1)

        # --- sobel / edge map (all fp32) ---
        gxp = pstat.tile([128, 128], f32, name="gxp", tag="gxp")
        gyp = pstat.tile([128, 128], f32, name="gyp", tag="gyp")
        nc.vector.tensor_tensor(out=gxp[:], in0=xmp[:, 2:130], in1=xmp[:, 0:128], op=ALU.subtract)
        nc.vector.scalar_tensor_tensor(out=gyp[:], in0=xmp[:, 1:129], scalar=2.0,
                                       in1=xmp[:, 0:128], op0=ALU.mult, op1=ALU.add)
        nc.vector.tensor_tensor(out=gyp[:], in0=gyp[:], in1=xmp[:, 2:130], op=ALU.add)
        psx = ppsm.tile([128, 128], f32, name="psx", tag="sm")
        nc.tensor.matmul(psx[:], lhsT=bsbd_s[:, 0:128], rhs=gxp[:], start=True, stop=True)
        psy = ppsm.tile([128, 128], f32, name="psy", tag="sm")
        nc.tensor.matmul(psy[:], lhsT=bsbd_s[:, 128:256], rhs=gyp[:], start=True, stop=True)
        mag = pstat.tile([128, 128], f32, name="mag", tag="mag")
        m2 = pstat.tile([128, 128], f32, name="m2t", tag="m2t")
        nc.scalar.activation(mag[:], psx[:], AF.Square)
        nc.scalar.activation(m2[:], psy[:], AF.Square)
        nc.vector.tensor_tensor(out=mag[:], in0=mag[:], in1=m2[:], op=ALU.add)
        nc.scalar.activation(mag[:], mag[:], AF.Sqrt, bias=c_eps8[:], scale=1.0)
        mx = pstat.tile([128, 1], f32, name="mx", tag="mx")
        mn = pstat.tile([128, 1], f32, name="mn", tag="mn")
        nc.vector.tensor_reduce(mx[:], mag[:], axis=AX.X, op=ALU.max)
        nc.vector.tensor_reduce(mn[:], mag[:], axis=AX.X, op=ALU.min)
        mxs = pstat.tile([1, 1], f32, name="mxs", tag="mxs")
        nmn = pstat.tile([1, 1], f32, name="nmn", tag="nmn")
        mneg = pstat.tile([128, 1], f32, name="mneg", tag="mneg")
        nc.vector.tensor_scalar_mul(mneg[:], mn[:], -1.0)
        nc.gpsimd.tensor_reduce(mxs[:], mx[:], axis=AX.C, op=ALU.max)
        nc.gpsimd.tensor_reduce(nmn[:], mneg[:], axis=AX.C, op=ALU.max)  # = -emin
        den = pstat.tile([1, 1], f32, name="den", tag="den")
        nc.vector.tensor_tensor(out=den[:], in0=mxs[:], in1=nmn[:], op=ALU.add)
        nc.vector.tensor_scalar_add(den[:], den[:], 1e-8)
        rec = pstat.tile([1, 1], f32, name="rec", tag="rec")
        nc.vector.reciprocal(rec[:], den[:])
        be = pstat.tile([1, 1], f32, name="be", tag="be")
        nc.vector.tensor_tensor(out=be[:], in0=nmn[:], in1=rec[:], op=ALU.mult)
        reca = pstat.tile([128, 1], f32, name="reca", tag="reca")
        bea = pstat.tile([128, 1], f32, name="bea", tag="bea")
        nc.gpsimd.partition_broadcast(reca[:], rec[:])
        nc.gpsimd.partition_broadcast(bea[:], be[:])
        nc.scalar.activation(edge_f[:], mag[:], AF.Identity, scale=reca[:], bias=bea[:])
        nc.sync.dma_start(edge_o[:, :], edge_f[:])
        nc.vector.tensor_copy(edge_b[:], edge_f[:])
        nc.sync.dma_start(eflat_sp[0:1, :], edge_b[:])

        # ================ P2a: gate conv (uses fz + edge9) ================
        pg1 = tc.alloc_tile_pool(name="pg1", bufs=1)
        g1n = pg1.tile([64, LC], bf16, name="g1n", tag="g1n")
        zero_pads(nc, g1n)

        pe9 = tc.alloc_tile_pool(name="pe9", bufs=1, side="right")
        e9 = pe9.tile([9, LC], bf16, name="e9", tag="e9")
        nc.gpsimd.memset(e9[:], 0.0)
        for t, (ty, tx) in enumerate(TAPS):
            base_t = (1 - ty) * CW + (1 - tx)
            dst = e9[t:t + 1, base_t:base_t + H * CW].rearrange("p (r x) -> p r x", r=H)[:, :, 0:W]
            nc.sync.dma_start(dst, edge_b[:])

        scr64 = pstat.tile([64, NPT], f32, name="scr64", tag="scr64")
        for ti in range(NT):
            pt = pps.tile([64, NPT], f32, name=f"g1p{ti}", tag="acc")
            for c in range(2):
                for t in range(9):
                    col0 = (t * 2 + c) * 64
                    nc.tensor.matmul(pt[:], lhsT=wg1_s[:, col0:col0 + 64],
                                     rhs=win(fz[c], ti, TAPS[t][0] * CW + TAPS[t][1]),
                                     start=(c == 0 and t == 0), stop=False)
            nc.tensor.matmul(pt[:], lhsT=wg1e_s[:], rhs=win(e9, ti, 0, parts=9),
                             start=False, stop=True)
            nc.scalar.activation(win(g1n, ti, 0), pt[:], AF.Copy,
                                 accum_out=g1sum[:, ti:ti + 1])
            nc.scalar.activation(scr64[:], pt[:], AF.Square,
                                 accum_out=g1sq[:, ti:ti + 1])
        pe9.release()

        ag, bg = gn_finalize(64, 8, [g1sum], [g1sq], gmat8_s, gnwg_s, gnbg_s,
                             8 * HW, "g")
        for ti in range(NT):
            nc.scalar.activation(win(g1n, ti, 0), win(g1n, ti, 0), AF.Gelu,
                                 scale=ag[:, 0:1], bias=bg[:, 0:1])

        # gate2 1x1 conv + sigmoid -> gate_o + scaled bf16 gate spill
        pgsb = tc.alloc_tile_pool(name="pgsb", bufs=1)
        gsb = pgsb.tile([1, HW], bf16, name="gsb", tag="gsb")
        for ti in range(NT):
            pt1 = ppsm.tile([1, NPT], f32, name=f"g2p{ti}", tag="sm")
            nc.tensor.matmul(pt1[:], lhsT=wg2_s[:], rhs=win(g1n, ti, 0),
                             start=True, stop=True)
            s1 = pstage.tile([1, NPT], f32, name=f"g2s{ti}", tag="st1", bufs=2)
            nc.scalar.activation(s1[:], pt1[:], AF.Sigmoid, bias=b2s_s[:], scale=1.0)
            nc.sync.dma_start(gate_o[0:1, ti * NPT:(ti + 1) * NPT], s1[:])
            nc.vector.tensor_scalar(out=gsb[:, ti * NPT:(ti + 1) * NPT], in0=s1[:],
                                    scalar1=rss_s[:], scalar2=None, op0=ALU.mult)
        nc.sync.dma_start(gspill[0:1, :], gsb[:])
        pgsb.release()
        pg1.release()
        pfz.release()

        # ================ P2b: hf1 conv (reload hf canvases) ================
        phf = tc.alloc_tile_pool(name="phf", bufs=1, side="right")
        hfc = [phf.tile([128, LC], bf16, name=f"hfc{c}", tag=f"hfc{c}") for c in range(2)]
        NSPL = 8
        for c in range(2):
            for qq in range(NSPL):
                a0, a1 = qq * (LC // NSPL), (qq + 1) * (LC // NSPL)
                nc.sync.dma_start(hfc[c][:, a0:a1], hfsp[c][:, a0:a1])
        ph1 = tc.alloc_tile_pool(name="ph1", bufs=1)
        h1n = ph1.tile([128, LC], bf16, name="h1n", tag="h1n")
        zero_pads(nc, h1n)

        scr128 = pstat.tile([128, NPT], f32, name="scr128", tag="scr128")
        for ti in range(NT):
            pt = pps.tile([128, NPT], f32, name=f"h1p{ti}", tag="acc")
            k = 0
            for c in range(2):
                for t in range(9):
                    col0 = (t * 2 + c) * 128
                    nc.tensor.matmul(pt[:], lhsT=whf1_s[:, col0:col0 + 128],
                                     rhs=win(hfc[c], ti, TAPS[t][0] * CW + TAPS[t][1]),
                                     start=(k == 0), stop=(k == 17))
                    k += 1
            nc.scalar.activation(win(h1n, ti, 0), pt[:], AF.Copy,
                                 accum_out=h1sum[:, ti:ti + 1])
            nc.scalar.activation(scr128[:], pt[:], AF.Square,
                                 accum_out=h1sq[:, ti:ti + 1])
        phf.release()

        ah, bh = gn_finalize(128, 16, [h1sum], [h1sq], gmat16_s, gnwh_s, gnbh_s,
                             16 * HW, "h")
        for ti in range(NT):
            nc.scalar.activation(win(h1n, ti, 0), win(h1n, ti, 0), AF.Gelu,
                                 scale=ah[:, 0:1], bias=bh[:, 0:1])

        # ================ P3: hf2 conv -> rin = fused + edge*hf_feat ================
        prin = tc.alloc_tile_pool(name="prin", bufs=1, side="right")
        rin = [prin.tile([128, LC], bf16, name=f"rin{c}", tag=f"rin{c}") for c in range(2)]
        pedge = tc.alloc_tile_pool(name="pedge", bufs=1, side="right")
        ebc = pedge.tile([128, HW], bf16, name="ebc", tag="ebc")
        pesb = tc.alloc_tile_pool(name="pesb", bufs=2, side="right")
        QB = 1024
        for q in range(HW // QB):
            esb = pesb.tile([1, QB], bf16, name=f"esb{q}", tag="esb")
            nc.sync.dma_start(esb[:], eflat_sp[0:1, q * QB:(q + 1) * QB])
            nc.gpsimd.partition_broadcast(ebc[:, q * QB:(q + 1) * QB], esb[:])
        pesb.release()
        zero_pads(nc, rin[0])
        zero_pads(nc, rin[1])

        for ti in range(NT):
            for m in range(2):
                pt = pps.tile([128, NPT], f32, name=f"h2p{ti}_{m}", tag="acc")
                for t in range(9):
                    col0 = (t * 2 + m) * 128
                    nc.tensor.matmul(pt[:], lhsT=whf2_s[:, col0:col0 + 128],
                                     rhs=win(h1n, ti, TAPS[t][0] * CW + TAPS[t][1]),
                                     start=(t == 0), stop=(t == 8))
                t1 = pstage.tile([128, NPT], f32, name=f"h2t{ti}_{m}", tag="stf32b", bufs=2)
                nc.vector.tensor_tensor(out=t1[:], in0=pt[:],
                                        in1=ebc[:, ti * NPT:(ti + 1) * NPT], op=ALU.mult)
                fb = pstage.tile([128, NPT], bf16, name=f"h2f{ti}_{m}", tag="stbf", bufs=6)
                nc.sync.dma_start(fb[:], fzlin[m][:, ti * NPT:(ti + 1) * NPT])
                nc.vector.tensor_tensor(out=win(rin[m], ti, 0), in0=t1[:], in1=fb[:], op=ALU.add)
        ph1.release()
        pedge.release()

        # ================ P4: ref1 conv ================
        pr1 = tc.alloc_tile_pool(name="pr1", bufs=1)
        r1n = [pr1.tile([128, LC], bf16, name=f"r1n{c}", tag=f"r1n{c}") for c in range(2)]
        zero_pads(nc, r1n[0])
        zero_pads(nc, r1n[1])

        for ti in range(NT):
            for m in range(2):
                pt = pps.tile([128, NPT], f32, name=f"r1p{ti}_{m}", tag="acc")
                k = 0
                for c in range(2):
                    for t in range(9):
                        col0 = ((t * 2 + c) * 2 + m) * 128
                        nc.tensor.matmul(pt[:], lhsT=wr1_s[:, col0:col0 + 128],
                                         rhs=win(rin[c], ti, TAPS[t][0] * CW + TAPS[t][1]),
                                         start=(k == 0), stop=(k == 17))
                        k += 1
                nc.scalar.activation(win(r1n[m], ti, 0), pt[:], AF.Copy,
                                     accum_out=r1sum[m][:, ti:ti + 1])
                nc.scalar.activation(scr128[:], pt[:], AF.Square,
                                     accum_out=r1sq[m][:, ti:ti + 1])
        prin.release()

        ar, br = gn_finalize(128, 32, r1sum, r1sq, gmat32_s, gnwr_s, gnbr_s,
                             32 * HW, "r")
        for ti in range(NT):
            for m in range(2):
                nc.scalar.activation(win(r1n[m], ti, 0), win(r1n[m], ti, 0), AF.Gelu,
                                     scale=ar[:, m:m + 1], bias=br[:, m:m + 1])

        # ================ P5: ref2 conv -> refined ================
        pgb = tc.alloc_tile_pool(name="pgb", bufs=1)
        gbc = pgb.tile([128, HW], bf16, name="gbc", tag="gbc")
        pgsb2 = tc.alloc_tile_pool(name="pgsb2", bufs=2)
        QB = 1024
        for q in range(HW // QB):
            gsb2 = pgsb2.tile([1, QB], bf16, name=f"gsb2{q}", tag="gsb2")
            nc.sync.dma_start(gsb2[:], gspill[0:1, q * QB:(q + 1) * QB])
            nc.gpsimd.partition_broadcast(gbc[:, q * QB:(q + 1) * QB], gsb2[:])
        pgsb2.release()

        for ti in range(NT):
            for m in range(2):
                pt = pps.tile([128, NPT], f32, name=f"r2p{ti}_{m}", tag="acc")
                k = 0
                for c in range(2):
                    for t in range(9):
                        col0 = ((t * 2 + c) * 2 + m) * 128
                        nc.tensor.matmul(pt[:], lhsT=wr2_s[:, col0:col0 + 128],
                                         rhs=win(r1n[c], ti, TAPS[t][0] * CW + TAPS[t][1]),
                                         start=(k == 0), stop=(k == 17))
                        k += 1
                t1 = pstage.tile([128, NPT], f32, name=f"r2t{ti}_{m}", tag="stf32b", bufs=2)
                nc.vector.tensor_tensor(out=t1[:], in0=pt[:],
                                        in1=gbc[:, ti * NPT:(ti + 1) * NPT], op=ALU.mult)
                ff = pstage.tile([128, NPT], f32, name=f"r2f{ti}_{m}", tag="stf32", bufs=6)
                nc.sync.dma_start(ff[:], fused[m * 128:(m + 1) * 128, ti * NPT:(ti + 1) * NPT])
                ot = pstage.tile([128, NPT], f32, name=f"r2o{ti}_{m}", tag="stf32c", bufs=2)
                nc.vector.tensor_tensor(out=ot[:], in0=t1[:], in1=ff[:], op=ALU.add)
                nc.sync.dma_start(refined[m * 128:(m + 1) * 128, ti * NPT:(ti + 1) * NPT], ot[:])
        pgb.release()
        pr1.release()

    nc.compile()
    _CACHE["built"] = nc
    return nc


# ---------------- host orchestration ----------------
def _prep_inputs(inputs):
    f32 = np.float32
    consts = _host_consts()
    shared = {
        "whf1": _pack_conv(np.asarray(inputs["hf_w1"], f32), 2, 1, 128),
        "whf2": _pack_conv(np.asarray(inputs["hf_w2"], f32), 1, 2, 128),
        "wg1": _pack_conv(np.asarray(inputs["gate_w1"], f32)[:, :256], 2, 1, 64),
        "wg1e": np.ascontiguousarray(
            np.asarray(inputs["gate_w1"], f32)[:, 256, :, :].reshape(64, 9).T
        ).astype(ml_dtypes.bfloat16),
        "wg2": np.asarray(inputs["gate_w2"], f32).reshape(1, 64).T.astype(ml_dtypes.bfloat16).copy(),
        "wr1": _pack_conv(np.asarray(inputs["ref_w1"], f32), 2, 2, 128),
        "wr2": _pack_conv(np.asarray(inputs["ref_w2"], f32), 2, 2, 128),
        "bsbd": consts["bsbd"],
        "gmat16": consts["gmat16"], "gmat32": consts["gmat32"], "gmat8": consts["gmat8"],
        "idlap": consts["idlap"], "ones1": consts["ones1"],
        "gnwh": np.asarray(inputs["hf_gn_w"], f32).reshape(128, 1),
        "gnbh": np.asarray(inputs["hf_gn_b"], f32).reshape(128, 1),
        "gnwg": np.asarray(inputs["gate_gn_w"], f32).reshape(64, 1),
        "gnbg": np.asarray(inputs["gate_gn_b"], f32).reshape(64, 1),
        "gnwr": np.asarray(inputs["ref_gn_w"], f32).reshape(2, 128).T.copy(),
        "gnbr": np.asarray(inputs["ref_gn_b"], f32).reshape(2, 128).T.copy(),
        "b2s": np.asarray(inputs["gate_b2"], f32).reshape(1, 1),
        "rss": np.asarray(inputs["residual_scale"], f32).reshape(1, 1),
    }
    fused = np.asarray(inputs["fused"], f32)
    per_core = []
    for b in range(8):
        m = dict(shared)
        m["fused"] = np.ascontiguousarray(fused[b].reshape(256, HW))
        per_core.append(m)
    return per_core


def _postprocess(results):
    refined = np.stack([r["refined"].reshape(256, H, W) for r in results]).astype(np.float32)
    edge = np.stack([r["edge_o"].reshape(1, H, W) for r in results]).astype(np.float32)
    gate = np.stack([r["gate_o"].reshape(-1) for r in results])
    gate_mean = np.float32(np.mean(gate.astype(np.float64)))
    return refined, edge, gate_mean


def kernel(**inputs):
    nc = _build()
    per_core = _prep_inputs(inputs)
    from concourse.bass_utils import run_bass_kernel_spmd
    res = run_bass_kernel_spmd(nc, per_core, list(range(8)))
    return _postprocess(res.results)


# ---------------- single-core simulator check (used by test.py) ----------------
def sim_one_core(inputs, core=0):
    import concourse.bass_interp as bass_interp
    nc = _build()
    per_core = _prep_inputs(inputs)
    sim = bass_interp.CoreSim(nc)
    for k, v in per_core[core].items():
        sim.tensor(k)[:] = v
    sim.simulate()
    out = {k: np.array(sim.tensor(k)) for k in ("refined", "edge_o", "gate_o")}
    return out


# revision 26
# speedup vs baseline: 1.1989x; 1.1324x over previous
"""Trainium2 Bass kernel for BoundaryRefinementModule.

One image per NeuronCore (B=8 over 8 cores, pure data parallel).
Convs are 9-shifted-matmuls accumulating in PSUM; activations in bf16,
accumulation + stats + pointwise in fp32.

Layout: activations live in SBUF "canvases" of shape [128, 16904] bf16,
where pixel (h, w) of a 128x128 image sits at free index (h+1)*130+(w+1)
(a 130x130 zero-padded canvas, flattened; +4 tail pad so conv windows
never slice out of range).
"""

import numpy as np
import ml_dtypes

# ---------------- geometry ----------------
H = W = 128
HW = H * W
CW = 130                  # canvas row width
LC = 16904                # canvas free elems (130*130 + 4 tail)
ORG = CW + 1              # canvas index of pixel (0,0)
RT = 4                    # image rows per spatial tile
NT = H // RT              # 32 spatial tiles
NPT = RT * W              # 512 pixels per tile
TAPS = [(ty, tx) for ty in (-1, 0, 1) for tx in (-1, 0, 1)]
LAP_TAPS = [((-1, 0), 1.0), ((0, -1), 1.0), ((0, 0), -4.0), ((0, 1), 1.0), ((1, 0), 1.0)]
EPS_GN = 1e-5

_CACHE = {}


# ---------------- host-side weight packing ----------------
def _pack_conv(w, cin_chunks, cout_chunks, mc):
    """w: (Cout, Cin, 3, 3) fp32 -> (128, 9*cin_chunks*cout_chunks*mc) bf16.

    Column block for (tap t, cin chunk c, cout chunk m):
      col0 = ((t*cin_chunks + c)*cout_chunks + m)*mc
      A[k, col0+j] = w[m*mc+j, c*128+k, ty+1, tx+1]
    """
    ncol = 9 * cin_chunks * cout_chunks * mc
    A = np.zeros((128, ncol), np.float32)
    for t, (ty, tx) in enumerate(TAPS):
        for c in range(cin_chunks):
            for m in range(cout_chunks):
                col0 = ((t * cin_chunks + c) * cout_chunks + m) * mc
                blk = w[m * mc:(m + 1) * mc, c * 128:(c + 1) * 128, ty + 1, tx + 1]
                A[:, col0:col0 + mc] = blk.T
    return A.astype(ml_dtypes.bfloat16)


def _host_consts():
    # Vertical Sobel band matrices (SAME zero boundary), fp32.
    Bs = np.zeros((128, 128), np.float32)   # smoothing [1,2,1]
    Bd = np.zeros((128, 128), np.float32)   # difference [-1,0,1]
    for h in range(128):
        for d in (-1, 0, 1):
            k = h + d
            if 0 <= k < 128:
                Bs[k, h] = (1.0, 2.0, 1.0)[d + 1]
                Bd[k, h] = float(d)
    bsbd = np.concatenate([Bs, Bd], axis=1)  # (128, 256)

    def block_ind(p, g):
        M = np.zeros((p, p), np.float32)
        for i in range(p):
            M[i, (i // g) * g:(i // g + 1) * g] = 1.0
        return M

    gmat16 = block_ind(128, 16)   # hf GN: 128 ch, groups of 16
    gmat32 = block_ind(128, 32)   # ref GN: per-chunk groups of 32
    gmat8 = block_ind(64, 8)      # gate GN: 64 ch, groups of 8

    idlap = np.zeros((128, 256), np.float32)
    idlap[:, 0:128] = np.eye(128)
    idlap[:, 128:256] = -4.0 * np.eye(128)
    ones1 = np.full((128, 1), 1.0 / 256.0, np.float32)  # channel-mean weights
    return {
        "bsbd": bsbd.astype(np.float32),
        "gmat16": gmat16, "gmat32": gmat32, "gmat8": gmat8,
        "idlap": idlap.astype(ml_dtypes.bfloat16),
        "ones1": ones1.astype(np.float32),
    }


# ---------------- bass program ----------------
def _build():
    if "built" in _CACHE:
        return _CACHE["built"]
    from contextlib import ExitStack
    import concourse.bacc as bacc
    import concourse.tile as tile
    from concourse import mybir
    import concourse.bass as bass

    f32 = mybir.dt.float32
    bf16 = mybir.dt.bfloat16
    AF = mybir.ActivationFunctionType
    ALU = mybir.AluOpType
    AX = mybir.AxisListType

    nc = bacc.Bacc("TRN2", target_bir_lowering=False, debug=False, num_devices=8)

    # -------- DRAM I/O --------
    fused = nc.dram_tensor("fused", [256, HW], f32, kind="ExternalInput").ap()
    whf1 = nc.dram_tensor("whf1", [128, 2304], bf16, kind="ExternalInput").ap()
    whf2 = nc.dram_tensor("whf2", [128, 2304], bf16, kind="ExternalInput").ap()
    wg1 = nc.dram_tensor("wg1", [128, 1152], bf16, kind="ExternalInput").ap()
    wg1e = nc.dram_tensor("wg1e", [9, 64], bf16, kind="ExternalInput").ap()
    wg2 = nc.dram_tensor("wg2", [64, 1], bf16, kind="ExternalInput").ap()
    wr1 = nc.dram_tensor("wr1", [128, 4608], bf16, kind="ExternalInput").ap()
    wr2 = nc.dram_tensor("wr2", [128, 4608], bf16, kind="ExternalInput").ap()
    bsbd_d = nc.dram_tensor("bsbd", [128, 256], f32, kind="ExternalInput").ap()
    gmat16_d = nc.dram_tensor("gmat16", [128, 128], f32, kind="ExternalInput").ap()
    gmat32_d = nc.dram_tensor("gmat32", [128, 128], f32, kind="ExternalInput").ap()
    gmat8_d = nc.dram_tensor("gmat8", [64, 64], f32, kind="ExternalInput").ap()
    idlap_d = nc.dram_tensor("idlap", [128, 256], bf16, kind="ExternalInput").ap()
    ones1_d = nc.dram_tensor("ones1", [128, 1], f32, kind="ExternalInput").ap()
    gnwh_d = nc.dram_tensor("gnwh", [128, 1], f32, kind="ExternalInput").ap()
    gnbh_d = nc.dram_tensor("gnbh", [128, 1], f32, kind="ExternalInput").ap()
    gnwg_d = nc.dram_tensor("gnwg", [64, 1], f32, kind="ExternalInput").ap()
    gnbg_d = nc.dram_tensor("gnbg", [64, 1], f32, kind="ExternalInput").ap()
    gnwr_d = nc.dram_tensor("gnwr", [128, 2], f32, kind="ExternalInput").ap()
    gnbr_d = nc.dram_tensor("gnbr", [128, 2], f32, kind="ExternalInput").ap()
    b2s_d = nc.dram_tensor("b2s", [1, 1], f32, kind="ExternalInput").ap()
    rss_d = nc.dram_tensor("rss", [1, 1], f32, kind="ExternalInput").ap()

    refined = nc.dram_tensor("refined", [256, HW], f32, kind="ExternalOutput").ap()
    edge_o = nc.dram_tensor("edge_o", [128, 128], f32, kind="ExternalOutput").ap()
    gate_o = nc.dram_tensor("gate_o", [1, HW], f32, kind="ExternalOutput").ap()



    def win(cv, ti, off, parts=None):
        """Conv rhs / write window: (P, RT, 128), canvas row stride CW."""
        base = (ti * RT + 1) * CW + 1 + off
        w = cv[:, base:base + RT * CW].rearrange("p (r x) -> p r x", r=RT)
        return w[:, :, 0:W]

    def interior(cv):
        w = cv[:, CW:CW + H * CW].rearrange("p (r x) -> p r x", r=H)
        return w[:, :, 1:1 + W]

    def zero_pads(nc, cv):
        # top row + col0 of row 1; the col129/col0 pair between rows; bottom row + tail
        nc.vector.memset(cv[:, 0:CW + 1], 0.0)
        mid = cv[:, CW + W + 1:CW + W + 1 + 127 * CW].rearrange(
            "p (r x) -> p r x", r=127)[:, :, 0:2]
        nc.vector.memset(mid, 0.0)
        nc.vector.memset(cv[:, LC - CW - 5:LC], 0.0)

    with tile.TileContext(nc) as tc, ExitStack() as CTX:
        # ---------------- persistent pools ----------------
        pw = CTX.enter_context(tc.tile_pool(name="pw", bufs=1))
        pstat = CTX.enter_context(tc.tile_pool(name="pstat", bufs=1))
        pstage = CTX.enter_context(tc.tile_pool(name="pstage", bufs=1))
        pps = CTX.enter_context(tc.tile_pool(name="pps", bufs=6, space="PSUM"))
        ppsm = CTX.enter_context(tc.tile_pool(name="ppsm", bufs=2, space="PSUM"))
        pdram = CTX.enter_context(tc.tile_pool(name="pdram", bufs=1, space="DRAM"))

        # DRAM spill tiles (tracked by Tile for DMA ordering)
        hfsp = [pdram.tile([128, HW], bf16, name=f"hfsp{c}", tag=f"hfsp{c}") for c in range(2)]
        fzlin = [pdram.tile([128, HW], bf16, name=f"fzlin{c}", tag=f"fzlin{c}") for c in range(2)]
        eflat_sp = pdram.tile([1, HW], bf16, name="eflat_sp", tag="eflat_sp")
        gspill = pdram.tile([1, HW], bf16, name="gspill", tag="gspill")

        # weights / consts to SBUF
        def load(name, src, shape, dt):
            t = pw.tile(shape, dt, name=name, tag=name)
            nc.sync.dma_start(t[:], src)
            return t

        whf1_s = load("whf1s", whf1, [128, 2304], bf16)
        whf2_s = load("whf2s", whf2, [128, 2304], bf16)
        wg1_s = load("wg1s", wg1, [128, 1152], bf16)
        wg1e_s = load("wg1es", wg1e, [9, 64], bf16)
        wg2_s = load("wg2s", wg2, [64, 1], bf16)
        wr1_s = load("wr1s", wr1, [128, 4608], bf16)
        wr2_s = load("wr2s", wr2, [128, 4608], bf16)
        bsbd_s = load("bsbds", bsbd_d, [128, 256], f32)
        gmat16_s = load("gmat16s", gmat16_d, [128, 128], f32)
        gmat32_s = load("gmat32s", gmat32_d, [128, 128], f32)
        gmat8_s = load("gmat8s", gmat8_d, [64, 64], f32)
        idlap_s = load("idlaps", idlap_d, [128, 256], bf16)
        ones1_s = load("ones1s", ones1_d, [128, 1], f32)
        gnwh_s = load("gnwhs", gnwh_d, [128, 1], f32)
        gnbh_s = load("gnbhs", gnbh_d, [128, 1], f32)
        gnwg_s = load("gnwgs", gnwg_d, [64, 1], f32)
        gnbg_s = load("gnbgs", gnbg_d, [64, 1], f32)
        gnwr_s = load("gnwrs", gnwr_d, [128, 2], f32)
        gnbr_s = load("gnbrs", gnbr_d, [128, 2], f32)
        b2s_s = load("b2ss", b2s_d, [1, 1], f32)
        rss_s = load("rsss", rss_d, [1, 1], f32)

        # small persistent stat tiles
        xmp = pstat.tile([128, CW], f32, name="xmp", tag="xmp")
        edge_f = pstat.tile([128, 128], f32, name="edge_f", tag="edge_f")
        edge_b = pstat.tile([128, 128], bf16, name="edge_b", tag="edge_b")
        h1sum = pstat.tile([128, NT], f32, name="h1sum", tag="h1sum")
        h1sq = pstat.tile([128, NT], f32, name="h1sq", tag="h1sq")
        g1sum = pstat.tile([64, NT], f32, name="g1sum", tag="g1sum")
        g1sq = pstat.tile([64, NT], f32, name="g1sq", tag="g1sq")
        r1sum = [pstat.tile([128, NT], f32, name=f"r1sum{c}", tag=f"r1sum{c}") for c in range(2)]
        r1sq = [pstat.tile([128, NT], f32, name=f"r1sq{c}", tag=f"r1sq{c}") for c in range(2)]

        nc.vector.memset(xmp[:], 0.0)

        c_epsgn = pstat.tile([128, 1], f32, name="c_epsgn", tag="c_epsgn")
        nc.vector.memset(c_epsgn[:], EPS_GN)
        c_eps8 = pstat.tile([128, 1], f32, name="c_eps8", tag="c_eps8")
        nc.vector.memset(c_eps8[:], 1e-8)

        # ---------------- GN finalize helper ----------------
        def gn_finalize(nparts, chans_per_col, sum_tiles, sq_tiles, gmat, gnw, gnb, npix, prefix):
            k = len(sum_tiles)
            chan = pstat.tile([nparts, 2 * k], f32, name=f"{prefix}chan", tag=f"{prefix}chan")
            for j, t in enumerate(sum_tiles):
                nc.vector.tensor_reduce(chan[:, j:j + 1], t[:], axis=AX.X, op=ALU.add)
            for j, t in enumerate(sq_tiles):
                nc.vector.tensor_reduce(chan[:, k + j:k + j + 1], t[:], axis=AX.X, op=ALU.add)
            gp = ppsm.tile([nparts, 2 * k], f32, name=f"{prefix}gp", tag="sm")
            nc.tensor.matmul(gp[:], lhsT=gmat[:], rhs=chan[:], start=True, stop=True)
            mv = pstat.tile([nparts, k], f32, name=f"{prefix}mv", tag=f"{prefix}mv")
            ex2 = pstat.tile([nparts, k], f32, name=f"{prefix}ex2", tag=f"{prefix}ex2")
            nc.vector.tensor_scalar_mul(mv[:], gp[:, 0:k], 1.0 / npix)
            nc.vector.tensor_scalar_mul(ex2[:], gp[:, k:2 * k], 1.0 / npix)
            var = pstat.tile([nparts, k], f32, name=f"{prefix}var", tag=f"{prefix}var")
            nc.vector.tensor_tensor(out=var[:], in0=mv[:], in1=mv[:], op=ALU.mult)
            nc.vector.tensor_tensor(out=var[:], in0=ex2[:], in1=var[:], op=ALU.subtract)
            sd = pstat.tile([nparts, k], f32, name=f"{prefix}sd", tag=f"{prefix}sd")
            nc.scalar.activation(sd[:], var[:], AF.Sqrt, bias=c_epsgn[0:nparts, :], scale=1.0)
            rinv = pstat.tile([nparts, k], f32, name=f"{prefix}rinv", tag=f"{prefix}rinv")
            nc.vector.reciprocal(rinv[:], sd[:])
            av = pstat.tile([nparts, k], f32, name=f"{prefix}av", tag=f"{prefix}av")
            bv = pstat.tile([nparts, k], f32, name=f"{prefix}bv", tag=f"{prefix}bv")
            nc.vector.tensor_tensor(out=av[:], in0=gnw[:], in1=rinv[:], op=ALU.mult)
            nc.vector.scalar_tensor_tensor(out=bv[:], in0=mv[:], scalar=-1.0, in1=av[:],
                                           op0=ALU.mult, op1=ALU.mult)
            nc.vector.tensor_tensor(out=bv[:], in0=bv[:], in1=gnb[:], op=ALU.add)
            return av, bv

        # ================ P1: load fused, xm, laplacian, sobel ================
        pfz = tc.alloc_tile_pool(name="pfz", bufs=1, side="right")
        fz = [pfz.tile([128, LC], bf16, name=f"fz{c}", tag=f"fz{c}") for c in range(2)]
        zero_pads(nc, fz[0])
        zero_pads(nc, fz[1])

        def lap_tile(c, ti):
            pt = pps.tile([128, NPT], f32, name=f"lap{c}_{ti}", tag="acc")
            for i, ((ty, tx), coef) in enumerate(LAP_TAPS):
                lw = idlap_s[:, 128:256] if coef == -4.0 else idlap_s[:, 0:128]
                nc.tensor.matmul(pt[:], lhsT=lw, rhs=win(fz[c], ti, ty * CW + tx),
                                 start=(i == 0), stop=(i == len(LAP_TAPS) - 1))
            hb = pstage.tile([128, NPT], bf16, name=f"lapb{c}_{ti}", tag="stbf", bufs=6)
            nc.scalar.activation(hb[:], pt[:], AF.Copy)
            nc.sync.dma_start(hfsp[c][:, ti * NPT:(ti + 1) * NPT], hb[:])

        for ti in range(NT):
            pxm = ppsm.tile([1, NPT], f32, name=f"pxm{ti}", tag="sm")
            for c in range(2):
                ft = pstage.tile([128, NPT], f32, name=f"p1f{ti}_{c}", tag="stf32", bufs=6)
                nc.sync.dma_start(ft[:], fused[c * 128:(c + 1) * 128, ti * NPT:(ti + 1) * NPT])
                bt = pstage.tile([128, NPT], bf16, name=f"p1b{ti}_{c}", tag="stbf", bufs=6)
                nc.vector.tensor_copy(bt[:], ft[:])
                nc.sync.dma_start(fzlin[c][:, ti * NPT:(ti + 1) * NPT], bt[:])
                nc.vector.tensor_copy(win(fz[c], ti, 0),
                                      ft[:].rearrange("p (r x) -> p r x", r=RT))
                nc.tensor.matmul(pxm[:], lhsT=ones1_s[:], rhs=ft[:],
                                 start=(c == 0), stop=(c == 1))
            s1 = pstage.tile([1, NPT], f32, name=f"p1s{ti}", tag="st1", bufs=2)
            nc.scalar.activation(s1[:], pxm[:], AF.Copy)
            nc.sync.dma_start(xmp[ti * RT:(ti + 1) * RT, 1:1 + W], s1[:])
            if ti >= 1:
                for c in range(2):
                    lap_tile(c, ti - 1)
        for c in range(2):
            lap_tile(c, NT - 1)

        # --- sobel / edge map (all fp32) ---
        gxp = pstat.tile([128, 128], f32, name="gxp", tag="gxp")
        gyp = pstat.tile([128, 128], f32, name="gyp", tag="gyp")
        nc.vector.tensor_tensor(out=gxp[:], in0=xmp[:, 2:130], in1=xmp[:, 0:128], op=ALU.subtract)
        nc.vector.scalar_tensor_tensor(out=gyp[:], in0=xmp[:, 1:129], scalar=2.0,
                                       in1=xmp[:, 0:128], op0=ALU.mult, op1=ALU.add)
        nc.vector.tensor_tensor(out=gyp[:], in0=gyp[:], in1=xmp[:, 2:130], op=ALU.add)
        psx = ppsm.tile([128, 128], f32, name="psx", tag="sm")
        nc.tensor.matmul(psx[:], lhsT=bsbd_s[:, 0:128], rhs=gxp[:], start=True, stop=True)
        psy = ppsm.tile([128, 128], f32, name="psy", tag="sm")
        nc.tensor.matmul(psy[:], lhsT=bsbd_s[:, 128:256], rhs=gyp[:], start=True, stop=True)
        mag = pstat.tile([128, 128], f32, name="mag", tag="mag")
        m2 = pstat.tile([128, 128], f32, name="m2t", tag="m2t")
        nc.scalar.activation(mag[:], psx[:], AF.Square)
        nc.scalar.activation(m2[:], psy[:], AF.Square)
        nc.vector.tensor_tensor(out=mag[:], in0=mag[:], in1=m2[:], op=ALU.add)
        nc.scalar.activation(mag[:], mag[:], AF.Sqrt, bias=c_eps8[:], scale=1.0)
        mx = pstat.tile([128, 1], f32, name="mx", tag="mx")
        mn = pstat.tile([128, 1], f32, name="mn", tag="mn")
        nc.vector.tensor_reduce(mx[:], mag[:], axis=AX.X, op=ALU.max)
        nc.vector.tensor_reduce(mn[:], mag[:], axis=AX.X, op=ALU.min)
        mxs = pstat.tile([1, 1], f32, name="mxs", tag="mxs")
        nmn = pstat.tile([1, 1], f32, name="nmn", tag="nmn")
        mneg = pstat.tile([128, 1], f32, name="mneg", tag="mneg")
        nc.vector.tensor_scalar_mul(mneg[:], mn[:], -1.0)
        nc.gpsimd.tensor_reduce(mxs[:], mx[:], axis=AX.C, op=ALU.max)
        nc.gpsimd.tensor_reduce(nmn[:], mneg[:], axis=AX.C, op=ALU.max)  # = -emin
        den = pstat.tile([1, 1], f32, name="den", tag="den")
        nc.vector.tensor_tensor(out=den[:], in0=mxs[:], in1=nmn[:], op=ALU.add)
        nc.vector.tensor_scalar_add(den[:], den[:], 1e-8)
        rec = pstat.tile([1, 1], f32, name="rec", tag="rec")
        nc.vector.reciprocal(rec[:], den[:])
        be = pstat.tile([1, 1], f32, name="be", tag="be")
        nc.vector.tensor_tensor(out=be[:], in0=nmn[:], in1=rec[:], op=ALU.mult)
        reca = pstat.tile([128, 1], f32, name="reca", tag="reca")
        bea = pstat.tile([128, 1], f32, name="bea", tag="bea")
        nc.gpsimd.partition_broadcast(reca[:], rec[:])
        nc.gpsimd.partition_broadcast(bea[:], be[:])
        nc.scalar.activation(edge_f[:], mag[:], AF.Identity, scale=reca[:], bias=bea[:])
        nc.sync.dma_start(edge_o[:, :], edge_f[:])
        nc.vector.tensor_copy(edge_b[:], edge_f[:])
        nc.sync.dma_start(eflat_sp[0:1, :], edge_b[:])

        # ================ P2a: gate conv (uses fz + edge9) ================
        pg1 = tc.alloc_tile_pool(name="pg1", bufs=1)
        g1n = pg1.tile([64, LC], bf16, name="g1n", tag="g1n")
        zero_pads(nc, g1n)

        pe9 = tc.alloc_tile_pool(name="pe9", bufs=1, side="right")
        e9 = pe9.tile([9, LC], bf16, name="e9", tag="e9")
        nc.vector.memset(e9[:], 0.0)
        for t, (ty, tx) in enumerate(TAPS):
            base_t = (1 - ty) * CW + (1 - tx)
            dst = e9[t:t + 1, base_t:base_t + H * CW].rearrange("p (r x) -> p r x", r=H)[:, :, 0:W]
            nc.sync.dma_start(dst, edge_b[:])

        scr64 = pstat.tile([64, NPT], f32, name="scr64", tag="scr64")
        for ti in range(NT):
            pt = pps.tile([64, NPT], f32, name=f"g1p{ti}", tag="acc")
            for c in range(2):
                for t in range(9):
                    col0 = (t * 2 + c) * 64
                    nc.tensor.matmul(pt[:], lhsT=wg1_s[:, col0:col0 + 64],
                                     rhs=win(fz[c], ti, TAPS[t][0] * CW + TAPS[t][1]),
                                     start=(c == 0 and t == 0), stop=False)
            nc.tensor.matmul(pt[:], lhsT=wg1e_s[:], rhs=win(e9, ti, 0, parts=9),
                             start=False, stop=True)
            nc.scalar.activation(win(g1n, ti, 0), pt[:], AF.Copy,
                                 accum_out=g1sum[:, ti:ti + 1])
            nc.scalar.activation(scr64[:], pt[:], AF.Square,
                                 accum_out=g1sq[:, ti:ti + 1])
        pe9.release()

        ag, bg = gn_finalize(64, 8, [g1sum], [g1sq], gmat8_s, gnwg_s, gnbg_s,
                             8 * HW, "g")
        for ti in range(NT):
            nc.scalar.activation(win(g1n, ti, 0), win(g1n, ti, 0), AF.Gelu,
                                 scale=ag[:, 0:1], bias=bg[:, 0:1])

        # gate2 1x1 conv + sigmoid -> gate_o + scaled bf16 gate spill
        pgsb = tc.alloc_tile_pool(name="pgsb", bufs=1)
        gsb = pgsb.tile([1, HW], bf16, name="gsb", tag="gsb")
        for ti in range(NT):
            pt1 = ppsm.tile([1, NPT], f32, name=f"g2p{ti}", tag="sm")
            nc.tensor.matmul(pt1[:], lhsT=wg2_s[:], rhs=win(g1n, ti, 0),
                             start=True, stop=True)
            s1 = pstage.tile([1, NPT], f32, name=f"g2s{ti}", tag="st1", bufs=2)
            nc.scalar.activation(s1[:], pt1[:], AF.Sigmoid, bias=b2s_s[:], scale=1.0)
            nc.sync.dma_start(gate_o[0:1, ti * NPT:(ti + 1) * NPT], s1[:])
            nc.vector.tensor_scalar(out=gsb[:, ti * NPT:(ti + 1) * NPT], in0=s1[:],
                                    scalar1=rss_s[:], scalar2=None, op0=ALU.mult)
        nc.sync.dma_start(gspill[0:1, :], gsb[:])
        pgsb.release()
        pg1.release()
        pfz.release()

        # ================ P2b: hf1 conv (reload hf canvases) ================
        phf = tc.alloc_tile_pool(name="phf", bufs=1, side="right")
        hfc = [phf.tile([128, LC], bf16, name=f"hfc{c}", tag=f"hfc{c}") for c in range(2)]
        zero_pads(nc, hfc[0])
        zero_pads(nc, hfc[1])
        for c in range(2):
            for ti in range(NT):
                hr = pstage.tile([128, NPT], bf16, name=f"hr{c}_{ti}", tag="stbf", bufs=6)
                nc.sync.dma_start(hr[:], hfsp[c][:, ti * NPT:(ti + 1) * NPT])
                nc.vector.tensor_copy(win(hfc[c], ti, 0),
                                      hr[:].rearrange("p (r x) -> p r x", r=RT))
        ph1 = tc.alloc_tile_pool(name="ph1", bufs=1)
        h1n = ph1.tile([128, LC], bf16, name="h1n", tag="h1n")
        zero_pads(nc, h1n)

        scr128 = pstat.tile([128, NPT], f32, name="scr128", tag="scr128")
        for ti in range(NT):
            pt = pps.tile([128, NPT], f32, name=f"h1p{ti}", tag="acc")
            k = 0
            for c in range(2):
                for t in range(9):
                    col0 = (t * 2 + c) * 128
                    nc.tensor.matmul(pt[:], lhsT=whf1_s[:, col0:col0 + 128],
                                     rhs=win(hfc[c], ti, TAPS[t][0] * CW + TAPS[t][1]),
                                     start=(k == 0), stop=(k == 17))
                    k += 1
            nc.scalar.activation(win(h1n, ti, 0), pt[:], AF.Copy,
                                 accum_out=h1sum[:, ti:ti + 1])
            nc.scalar.activation(scr128[:], pt[:], AF.Square,
                                 accum_out=h1sq[:, ti:ti + 1])
        phf.release()

        ah, bh = gn_finalize(128, 16, [h1sum], [h1sq], gmat16_s, gnwh_s, gnbh_s,
                             16 * HW, "h")
        for ti in range(NT):
            nc.scalar.activation(win(h1n, ti, 0), win(h1n, ti, 0), AF.Gelu,
                                 scale=ah[:, 0:1], bias=bh[:, 0:1])

        # ================ P3: hf2 conv -> rin = fused + edge*hf_feat ================
        prin = tc.alloc_tile_pool(name="prin", bufs=1, side="right")
        rin = [prin.tile([128, LC], bf16, name=f"rin{c}", tag=f"rin{c}") for c in range(2)]
        pedge = tc.alloc_tile_pool(name="pedge", bufs=1, side="right")
        ebc = pedge.tile([128, HW], bf16, name="ebc", tag="ebc")
        pesb = tc.alloc_tile_pool(name="pesb", bufs=2, side="right")
        QB = 1024
        for q in range(HW // QB):
            esb = pesb.tile([1, QB], bf16, name=f"esb{q}", tag="esb")
            nc.sync.dma_start(esb[:], eflat_sp[0:1, q * QB:(q + 1) * QB])
            nc.gpsimd.partition_broadcast(ebc[:, q * QB:(q + 1) * QB], esb[:])
        pesb.release()
        zero_pads(nc, rin[0])
        zero_pads(nc, rin[1])

        for ti in range(NT):
            for m in range(2):
                pt = pps.tile([128, NPT], f32, name=f"h2p{ti}_{m}", tag="acc")
                for t in range(9):
                    col0 = (t * 2 + m) * 128
                    nc.tensor.matmul(pt[:], lhsT=whf2_s[:, col0:col0 + 128],
                                     rhs=win(h1n, ti, TAPS[t][0] * CW + TAPS[t][1]),
                                     start=(t == 0), stop=(t == 8))
                t1 = pstage.tile([128, NPT], f32, name=f"h2t{ti}_{m}", tag="stf32b", bufs=2)
                nc.vector.tensor_tensor(out=t1[:], in0=pt[:],
                                        in1=ebc[:, ti * NPT:(ti + 1) * NPT], op=ALU.mult)
                fb = pstage.tile([128, NPT], bf16, name=f"h2f{ti}_{m}", tag="stbf", bufs=6)
                nc.sync.dma_start(fb[:], fzlin[m][:, ti * NPT:(ti + 1) * NPT])
                nc.vector.tensor_tensor(out=win(rin[m], ti, 0), in0=t1[:], in1=fb[:], op=ALU.add)
        ph1.release()
        pedge.release()

        # ================ P4: ref1 conv ================
        pr1 = tc.alloc_tile_pool(name="pr1", bufs=1)
        r1n = [pr1.tile([128, LC], bf16, name=f"r1n{c}", tag=f"r1n{c}") for c in range(2)]
        zero_pads(nc, r1n[0])
        zero_pads(nc, r1n[1])

        for ti in range(NT):
            for m in range(2):
                pt = pps.tile([128, NPT], f32, name=f"r1p{ti}_{m}", tag="acc")
                k = 0
                for c in range(2):
                    for t in range(9):
                        col0 = ((t * 2 + c) * 2 + m) * 128
                        nc.tensor.matmul(pt[:], lhsT=wr1_s[:, col0:col0 + 128],
                                         rhs=win(rin[c], ti, TAPS[t][0] * CW + TAPS[t][1]),
                                         start=(k == 0), stop=(k == 17))
                        k += 1
                nc.scalar.activation(win(r1n[m], ti, 0), pt[:], AF.Copy,
                                     accum_out=r1sum[m][:, ti:ti + 1])
                nc.scalar.activation(scr128[:], pt[:], AF.Square,
                                     accum_out=r1sq[m][:, ti:ti + 1])
        prin.release()

        ar, br = gn_finalize(128, 32, r1sum, r1sq, gmat32_s, gnwr_s, gnbr_s,
                             32 * HW, "r")
        for ti in range(NT):
            for m in range(2):
                nc.scalar.activation(win(r1n[m], ti, 0), win(r1n[m], ti, 0), AF.Gelu,
                                     scale=ar[:, m:m + 1], bias=br[:, m:m + 1])

        # ================ P5: ref2 conv -> refined ================
        pgb = tc.alloc_tile_pool(name="pgb", bufs=1)
        gbc = pgb.tile([128, HW], bf16, name="gbc", tag="gbc")
        pgsb2 = tc.alloc_tile_pool(name="pgsb2", bufs=2)
        QB = 1024
        for q in range(HW // QB):
            gsb2 = pgsb2.tile([1, QB], bf16, name=f"gsb2{q}", tag="gsb2")
            nc.sync.dma_start(gsb2[:], gspill[0:1, q * QB:(q + 1) * QB])
            nc.gpsimd.partition_broadcast(gbc[:, q * QB:(q + 1) * QB], gsb2[:])
        pgsb2.release()

        for ti in range(NT):
            for m in range(2):
                pt = pps.tile([128, NPT], f32, name=f"r2p{ti}_{m}", tag="acc")
                k = 0
                for c in range(2):
                    for t in range(9):
                        col0 = ((t * 2 + c) * 2 + m) * 128
                        nc.tensor.matmul(pt[:], lhsT=wr2_s[:, col0:col0 + 128],
                                         rhs=win(r1n[c], ti, TAPS[t][0] * CW + TAPS[t][1]),
                                         start=(k == 0), stop=(k == 17))
                        k += 1
                t1 = pstage.tile([128, NPT], f32, name=f"r2t{ti}_{m}", tag="stf32b", bufs=2)
                nc.vector.tensor_tensor(out=t1[:], in0=pt[:],
                                        in1=gbc[:, ti * NPT:(ti + 1) * NPT], op=ALU.mult)
                ff = pstage.tile([128, NPT], f32, name=f"r2f{ti}_{m}", tag="stf32", bufs=6)
                nc.sync.dma_start(ff[:], fused[m * 128:(m + 1) * 128, ti * NPT:(ti + 1) * NPT])
                ot = pstage.tile([128, NPT], f32, name=f"r2o{ti}_{m}", tag="stf32c", bufs=2)
                nc.vector.tensor_tensor(out=ot[:], in0=t1[:], in1=ff[:], op=ALU.add)
                nc.sync.dma_start(refined[m * 128:(m + 1) * 128, ti * NPT:(ti + 1) * NPT], ot[:])
        pgb.release()
        pr1.release()

    nc.compile()
    _CACHE["built"] = nc
    return nc


# ---------------- host orchestration ----------------
def _prep_inputs(inputs):
    f32 = np.float32
    consts = _host_consts()
    shared = {
        "whf1": _pack_conv(np.asarray(inputs["hf_w1"], f32), 2, 1, 128),
        "whf2": _pack_conv(np.asarray(inputs["hf_w2"], f32), 1, 2, 128),
        "wg1": _pack_conv(np.asarray(inputs["gate_w1"], f32)[:, :256], 2, 1, 64),
        "wg1e": np.ascontiguousarray(
            np.asarray(inputs["gate_w1"], f32)[:, 256, :, :].reshape(64, 9).T
        ).astype(ml_dtypes.bfloat16),
        "wg2": np.asarray(inputs["gate_w2"], f32).reshape(1, 64).T.astype(ml_dtypes.bfloat16).copy(),
        "wr1": _pack_conv(np.asarray(inputs["ref_w1"], f32), 2, 2, 128),
        "wr2": _pack_conv(np.asarray(inputs["ref_w2"], f32), 2, 2, 128),
        "bsbd": consts["bsbd"],
        "gmat16": consts["gmat16"], "gmat32": consts["gmat32"], "gmat8": consts["gmat8"],
        "idlap": consts["idlap"], "ones1": consts["ones1"],
        "gnwh": np.asarray(inputs["hf_gn_w"], f32).reshape(128, 1),
        "gnbh": np.asarray(inputs["hf_gn_b"], f32).reshape(128, 1),
        "gnwg": np.asarray(inputs["gate_gn_w"], f32).reshape(64, 1),
        "gnbg": np.asarray(inputs["gate_gn_b"], f32).reshape(64, 1),
        "gnwr": np.asarray(inputs["ref_gn_w"], f32).reshape(2, 128).T.copy(),
        "gnbr": np.asarray(inputs["ref_gn_b"], f32).reshape(2, 128).T.copy(),
        "b2s": np.asarray(inputs["gate_b2"], f32).reshape(1, 1),
        "rss": np.asarray(inputs["residual_scale"], f32).reshape(1, 1),
    }
    fused = np.asarray(inputs["fused"], f32)
    per_core = []
    for b in range(8):
        m = dict(shared)
        m["fused"] = np.ascontiguousarray(fused[b].reshape(256, HW))
        per_core.append(m)
    return per_core


def _postprocess(results):
    refined = np.stack([r["refined"].reshape(256, H, W) for r in results]).astype(np.float32)
    edge = np.stack([r["edge_o"].reshape(1, H, W) for r in results]).astype(np.float32)
    gate = np.stack([r["gate_o"].reshape(-1) for r in results])
    gate_mean = np.float32(np.mean(gate.astype(np.float64)))
    return refined, edge, gate_mean


def kernel(**inputs):
    nc = _build()
    per_core = _prep_inputs(inputs)
    from concourse.bass_utils import run_bass_kernel_spmd
    res = run_bass_kernel_spmd(nc, per_core, list(range(8)))
    return _postprocess(res.results)


# ---------------- single-core simulator check (used by test.py) ----------------
def sim_one_core(inputs, core=0):
    import concourse.bass_interp as bass_interp
    nc = _build()
    per_core = _prep_inputs(inputs)
    sim = bass_interp.CoreSim(nc)
    for k, v in per_core[core].items():
        sim.tensor(k)[:] = v
    sim.simulate()
    out = {k: np.array(sim.tensor(k)) for k in ("refined", "edge_o", "gate_o")}
    return out


# revision 31
# speedup vs baseline: 1.2643x; 1.0545x over previous
"""Trainium2 Bass kernel for BoundaryRefinementModule.

One image per NeuronCore (B=8 over 8 cores, pure data parallel).
Convs are 9-shifted-matmuls accumulating in PSUM; activations in bf16,
accumulation + stats + pointwise in fp32.

Layout: activations live in SBUF "canvases" of shape [128, 16904] bf16,
where pixel (h, w) of a 128x128 image sits at free index (h+1)*130+(w+1)
(a 130x130 zero-padded canvas, flattened; +4 tail pad so conv windows
never slice out of range).
"""

import numpy as np
import ml_dtypes

# ---------------- geometry ----------------
H = W = 128
HW = H * W
CW = 130                  # canvas row width
LC = 16904                # canvas free elems (130*130 + 4 tail)
ORG = CW + 1              # canvas index of pixel (0,0)
RT = 4                    # image rows per spatial tile
NT = H // RT              # 32 spatial tiles
NPT = RT * W              # 512 pixels per tile
TAPS = [(ty, tx) for ty in (-1, 0, 1) for tx in (-1, 0, 1)]
LAP_TAPS = [((-1, 0), 1.0), ((0, -1), 1.0), ((0, 0), -4.0), ((0, 1), 1.0), ((1, 0), 1.0)]
EPS_GN = 1e-5

_CACHE = {}


# ---------------- host-side weight packing ----------------
def _pack_conv(w, cin_chunks, cout_chunks, mc):
    """w: (Cout, Cin, 3, 3) fp32 -> (128, 9*cin_chunks*cout_chunks*mc) bf16.

    Column block for (tap t, cin chunk c, cout chunk m):
      col0 = ((t*cin_chunks + c)*cout_chunks + m)*mc
      A[k, col0+j] = w[m*mc+j, c*128+k, ty+1, tx+1]
    """
    ncol = 9 * cin_chunks * cout_chunks * mc
    A = np.zeros((128, ncol), np.float32)
    for t, (ty, tx) in enumerate(TAPS):
        for c in range(cin_chunks):
            for m in range(cout_chunks):
                col0 = ((t * cin_chunks + c) * cout_chunks + m) * mc
                blk = w[m * mc:(m + 1) * mc, c * 128:(c + 1) * 128, ty + 1, tx + 1]
                A[:, col0:col0 + mc] = blk.T
    return A.astype(ml_dtypes.bfloat16)


def _host_consts():
    # Vertical Sobel band matrices (SAME zero boundary), fp32.
    Bs = np.zeros((128, 128), np.float32)   # smoothing [1,2,1]
    Bd = np.zeros((128, 128), np.float32)   # difference [-1,0,1]
    for h in range(128):
        for d in (-1, 0, 1):
            k = h + d
            if 0 <= k < 128:
                Bs[k, h] = (1.0, 2.0, 1.0)[d + 1]
                Bd[k, h] = float(d)
    bsbd = np.concatenate([Bs, Bd], axis=1)  # (128, 256)

    def block_ind(p, g):
        M = np.zeros((p, p), np.float32)
        for i in range(p):
            M[i, (i // g) * g:(i // g + 1) * g] = 1.0
        return M

    gmat16 = block_ind(128, 16)   # hf GN: 128 ch, groups of 16
    gmat32 = block_ind(128, 32)   # ref GN: per-chunk groups of 32
    # gate GN with col-packed g1n: k in 0..127 maps to channel k%64, groups of 8
    gmat8 = np.zeros((128, 64), np.float32)
    for k in range(128):
        c = k % 64
        gmat8[k, (c // 8) * 8:(c // 8 + 1) * 8] = 1.0

    idlap = np.zeros((128, 256), np.float32)
    idlap[:, 0:128] = np.eye(128)
    idlap[:, 128:256] = -4.0 * np.eye(128)
    ones1 = np.full((128, 1), 1.0 / 256.0, np.float32)  # channel-mean weights
    return {
        "bsbd": bsbd.astype(np.float32),
        "gmat16": gmat16, "gmat32": gmat32, "gmat8": gmat8,
        "idlap": idlap.astype(ml_dtypes.bfloat16),
        "ones1": ones1.astype(np.float32),
    }


# ---------------- bass program ----------------
def _build():
    if "built" in _CACHE:
        return _CACHE["built"]
    from contextlib import ExitStack
    import concourse.bacc as bacc
    import concourse.tile as tile
    from concourse import mybir
    import concourse.bass as bass

    f32 = mybir.dt.float32
    bf16 = mybir.dt.bfloat16
    AF = mybir.ActivationFunctionType
    ALU = mybir.AluOpType
    AX = mybir.AxisListType

    nc = bacc.Bacc("TRN2", target_bir_lowering=False, debug=False, num_devices=8)

    # -------- DRAM I/O --------
    fused = nc.dram_tensor("fused", [256, HW], f32, kind="ExternalInput").ap()
    whf1 = nc.dram_tensor("whf1", [128, 2304], bf16, kind="ExternalInput").ap()
    whf2 = nc.dram_tensor("whf2", [128, 2304], bf16, kind="ExternalInput").ap()
    wg1 = nc.dram_tensor("wg1", [128, 1152], bf16, kind="ExternalInput").ap()
    wg1e = nc.dram_tensor("wg1e", [9, 64], bf16, kind="ExternalInput").ap()
    wg2 = nc.dram_tensor("wg2", [128, 1], bf16, kind="ExternalInput").ap()
    wr1 = nc.dram_tensor("wr1", [128, 4608], bf16, kind="ExternalInput").ap()
    wr2 = nc.dram_tensor("wr2", [128, 4608], bf16, kind="ExternalInput").ap()
    bsbd_d = nc.dram_tensor("bsbd", [128, 256], f32, kind="ExternalInput").ap()
    gmat16_d = nc.dram_tensor("gmat16", [128, 128], f32, kind="ExternalInput").ap()
    gmat32_d = nc.dram_tensor("gmat32", [128, 128], f32, kind="ExternalInput").ap()
    gmat8_d = nc.dram_tensor("gmat8", [128, 64], f32, kind="ExternalInput").ap()
    idlap_d = nc.dram_tensor("idlap", [128, 256], bf16, kind="ExternalInput").ap()
    ones1_d = nc.dram_tensor("ones1", [128, 1], f32, kind="ExternalInput").ap()
    gnwh_d = nc.dram_tensor("gnwh", [128, 1], f32, kind="ExternalInput").ap()
    gnbh_d = nc.dram_tensor("gnbh", [128, 1], f32, kind="ExternalInput").ap()
    gnwg_d = nc.dram_tensor("gnwg", [64, 1], f32, kind="ExternalInput").ap()
    gnbg_d = nc.dram_tensor("gnbg", [64, 1], f32, kind="ExternalInput").ap()
    gnwr_d = nc.dram_tensor("gnwr", [128, 2], f32, kind="ExternalInput").ap()
    gnbr_d = nc.dram_tensor("gnbr", [128, 2], f32, kind="ExternalInput").ap()
    b2s_d = nc.dram_tensor("b2s", [1, 1], f32, kind="ExternalInput").ap()
    rss_d = nc.dram_tensor("rss", [1, 1], f32, kind="ExternalInput").ap()

    refined = nc.dram_tensor("refined", [256, HW], f32, kind="ExternalOutput").ap()
    edge_o = nc.dram_tensor("edge_o", [128, 128], f32, kind="ExternalOutput").ap()
    gate_o = nc.dram_tensor("gate_o", [1, HW], f32, kind="ExternalOutput").ap()



    def win(cv, ti, off, parts=None):
        """Conv rhs / write window: (P, RT, 128), canvas row stride CW."""
        base = (ti * RT + 1) * CW + 1 + off
        w = cv[:, base:base + RT * CW].rearrange("p (r x) -> p r x", r=RT)
        return w[:, :, 0:W]

    def interior(cv):
        w = cv[:, CW:CW + H * CW].rearrange("p (r x) -> p r x", r=H)
        return w[:, :, 1:1 + W]

    def zero_pads(nc, cv):
        # top row + col0 of row 1; the col129/col0 pair between rows; bottom row + tail
        nc.vector.memset(cv[:, 0:CW + 1], 0.0)
        mid = cv[:, CW + W + 1:CW + W + 1 + 127 * CW].rearrange(
            "p (r x) -> p r x", r=127)[:, :, 0:2]
        nc.vector.memset(mid, 0.0)
        nc.vector.memset(cv[:, LC - CW - 5:LC], 0.0)

    with tile.TileContext(nc) as tc, ExitStack() as CTX:
        # ---------------- persistent pools ----------------
        pw = CTX.enter_context(tc.tile_pool(name="pw", bufs=1))
        pstat = CTX.enter_context(tc.tile_pool(name="pstat", bufs=1))
        pstage = CTX.enter_context(tc.tile_pool(name="pstage", bufs=1))
        pps = CTX.enter_context(tc.tile_pool(name="pps", bufs=6, space="PSUM"))
        ppsm = CTX.enter_context(tc.tile_pool(name="ppsm", bufs=2, space="PSUM"))
        pdram = CTX.enter_context(tc.tile_pool(name="pdram", bufs=1, space="DRAM"))

        # DRAM spill tiles (tracked by Tile for DMA ordering)
        hfsp = [pdram.tile([128, HW], bf16, name=f"hfsp{c}", tag=f"hfsp{c}") for c in range(2)]
        fzlin = [pdram.tile([128, HW], bf16, name=f"fzlin{c}", tag=f"fzlin{c}") for c in range(2)]
        eflat_sp = pdram.tile([1, HW], bf16, name="eflat_sp", tag="eflat_sp")
        gspill = pdram.tile([1, HW], bf16, name="gspill", tag="gspill")

        # weights / consts to SBUF
        def load(name, src, shape, dt):
            t = pw.tile(shape, dt, name=name, tag=name)
            nc.sync.dma_start(t[:], src)
            return t

        idlap_s = load("idlaps", idlap_d, [128, 256], bf16)
        ones1_s = load("ones1s", ones1_d, [128, 1], f32)
        bsbd_s = load("bsbds", bsbd_d, [128, 256], f32)
        gnwh_s = load("gnwhs", gnwh_d, [128, 1], f32)
        gnbh_s = load("gnbhs", gnbh_d, [128, 1], f32)
        gnwg_s = load("gnwgs", gnwg_d, [64, 1], f32)
        gnbg_s = load("gnbgs", gnbg_d, [64, 1], f32)
        gnwr_s = load("gnwrs", gnwr_d, [128, 2], f32)
        gnbr_s = load("gnbrs", gnbr_d, [128, 2], f32)
        b2s_s = load("b2ss", b2s_d, [1, 1], f32)
        rss_s = load("rsss", rss_d, [1, 1], f32)

        # small persistent stat tiles
        xmp = pstat.tile([128, CW], f32, name="xmp", tag="xmp")
        edge_f = pstat.tile([128, 128], f32, name="edge_f", tag="edge_f")
        edge_b = pstat.tile([128, 128], bf16, name="edge_b", tag="edge_b")
        h1sum = pstat.tile([128, NT], f32, name="h1sum", tag="h1sum")
        h1sq = pstat.tile([128, NT], f32, name="h1sq", tag="h1sq")
        g1sum = pstat.tile([128, NT // 2], f32, name="g1sum", tag="g1sum")
        g1sq = pstat.tile([128, NT // 2], f32, name="g1sq", tag="g1sq")
        r1sum = [pstat.tile([128, NT], f32, name=f"r1sum{c}", tag=f"r1sum{c}") for c in range(2)]
        r1sq = [pstat.tile([128, NT], f32, name=f"r1sq{c}", tag=f"r1sq{c}") for c in range(2)]

        nc.vector.memset(xmp[:], 0.0)

        c_epsgn = pstat.tile([128, 1], f32, name="c_epsgn", tag="c_epsgn")
        nc.vector.memset(c_epsgn[:], EPS_GN)
        c_eps8 = pstat.tile([128, 1], f32, name="c_eps8", tag="c_eps8")
        nc.vector.memset(c_eps8[:], 1e-8)

        # ---------------- GN finalize helper ----------------
        def gn_finalize(nparts, chans_per_col, sum_tiles, sq_tiles, gmat, gnw, gnb, npix, prefix):
            k = len(sum_tiles)
            chan = pstat.tile([nparts, 2 * k], f32, name=f"{prefix}chan", tag=f"{prefix}chan")
            for j, t in enumerate(sum_tiles):
                nc.vector.tensor_reduce(chan[:, j:j + 1], t[:], axis=AX.X, op=ALU.add)
            for j, t in enumerate(sq_tiles):
                nc.vector.tensor_reduce(chan[:, k + j:k + j + 1], t[:], axis=AX.X, op=ALU.add)
            gp = ppsm.tile([nparts, 2 * k], f32, name=f"{prefix}gp", tag="sm")
            nc.tensor.matmul(gp[:], lhsT=gmat[:], rhs=chan[:], start=True, stop=True)
            mv = pstat.tile([nparts, k], f32, name=f"{prefix}mv", tag=f"{prefix}mv")
            ex2 = pstat.tile([nparts, k], f32, name=f"{prefix}ex2", tag=f"{prefix}ex2")
            nc.vector.tensor_scalar_mul(mv[:], gp[:, 0:k], 1.0 / npix)
            nc.vector.tensor_scalar_mul(ex2[:], gp[:, k:2 * k], 1.0 / npix)
            var = pstat.tile([nparts, k], f32, name=f"{prefix}var", tag=f"{prefix}var")
            nc.vector.tensor_tensor(out=var[:], in0=mv[:], in1=mv[:], op=ALU.mult)
            nc.vector.tensor_tensor(out=var[:], in0=ex2[:], in1=var[:], op=ALU.subtract)
            sd = pstat.tile([nparts, k], f32, name=f"{prefix}sd", tag=f"{prefix}sd")
            nc.scalar.activation(sd[:], var[:], AF.Sqrt, bias=c_epsgn[0:nparts, :], scale=1.0)
            rinv = pstat.tile([nparts, k], f32, name=f"{prefix}rinv", tag=f"{prefix}rinv")
            nc.vector.reciprocal(rinv[:], sd[:])
            av = pstat.tile([nparts, k], f32, name=f"{prefix}av", tag=f"{prefix}av")
            bv = pstat.tile([nparts, k], f32, name=f"{prefix}bv", tag=f"{prefix}bv")
            nc.vector.tensor_tensor(out=av[:], in0=gnw[:], in1=rinv[:], op=ALU.mult)
            nc.vector.scalar_tensor_tensor(out=bv[:], in0=mv[:], scalar=-1.0, in1=av[:],
                                           op0=ALU.mult, op1=ALU.mult)
            nc.vector.tensor_tensor(out=bv[:], in0=bv[:], in1=gnb[:], op=ALU.add)
            return av, bv

        # ================ P1: load fused, xm, laplacian, sobel ================
        pfz = tc.alloc_tile_pool(name="pfz", bufs=1, side="right")
        fz = [pfz.tile([128, LC], bf16, name=f"fz{c}", tag=f"fz{c}") for c in range(2)]
        zero_pads(nc, fz[0])
        zero_pads(nc, fz[1])

        def lap_tile(c, ti):
            pt = pps.tile([128, NPT], f32, name=f"lap{c}_{ti}", tag="acc")
            for i, ((ty, tx), coef) in enumerate(LAP_TAPS):
                lw = idlap_s[:, 128:256] if coef == -4.0 else idlap_s[:, 0:128]
                nc.tensor.matmul(pt[:], lhsT=lw, rhs=win(fz[c], ti, ty * CW + tx),
                                 start=(i == 0), stop=(i == len(LAP_TAPS) - 1))
            hb = pstage.tile([128, NPT], bf16, name=f"lapb{c}_{ti}", tag="stbf", bufs=6)
            nc.scalar.activation(hb[:], pt[:], AF.Copy)
            nc.sync.dma_start(hfsp[c][:, ti * NPT:(ti + 1) * NPT], hb[:])

        for ti in range(NT):
            fts = []
            for c in range(2):
                ft = pstage.tile([128, NPT], f32, name=f"p1f{ti}_{c}", tag="stf32", bufs=6)
                nc.sync.dma_start(ft[:], fused[c * 128:(c + 1) * 128, ti * NPT:(ti + 1) * NPT])
                fts.append(ft)
                bt = pstage.tile([128, NPT], bf16, name=f"p1b{ti}_{c}", tag="stbf", bufs=6)
                nc.vector.tensor_copy(bt[:], ft[:])
                nc.sync.dma_start(fzlin[c][:, ti * NPT:(ti + 1) * NPT], bt[:])
                nc.vector.tensor_copy(win(fz[c], ti, 0),
                                      ft[:].rearrange("p (r x) -> p r x", r=RT))
            fsum = pstage.tile([128, NPT], f32, name=f"p1fs{ti}", tag="stf32b", bufs=2)
            nc.vector.tensor_tensor(out=fsum[:], in0=fts[0][:], in1=fts[1][:], op=ALU.add)
            pxm = ppsm.tile([1, NPT], f32, name=f"pxm{ti}", tag="sm")
            nc.tensor.matmul(pxm[:], lhsT=ones1_s[:], rhs=fsum[:], start=True, stop=True)
            s1 = pstage.tile([1, NPT], f32, name=f"p1s{ti}", tag="st1", bufs=2)
            nc.scalar.activation(s1[:], pxm[:], AF.Copy)
            nc.sync.dma_start(xmp[ti * RT:(ti + 1) * RT, 1:1 + W], s1[:])
            if ti >= 1:
                for c in range(2):
                    lap_tile(c, ti - 1)
        for c in range(2):
            lap_tile(c, NT - 1)

        # heavy weight loads (needed from P2a onward; emitted late to keep the
        # startup DMA queues clear for the first fused tiles)
        whf1_s = load("whf1s", whf1, [128, 2304], bf16)
        whf2_s = load("whf2s", whf2, [128, 2304], bf16)
        wg1_s = load("wg1s", wg1, [128, 1152], bf16)
        wg1e_s = load("wg1es", wg1e, [9, 64], bf16)
        wg2_s = load("wg2s", wg2, [128, 1], bf16)
        wr1_s = load("wr1s", wr1, [128, 4608], bf16)
        wr2_s = load("wr2s", wr2, [128, 4608], bf16)
        gmat16_s = load("gmat16s", gmat16_d, [128, 128], f32)
        gmat32_s = load("gmat32s", gmat32_d, [128, 128], f32)
        gmat8_s = load("gmat8s", gmat8_d, [128, 64], f32)

        # --- sobel / edge map (all fp32) ---
        gxp = pstat.tile([128, 128], f32, name="gxp", tag="gxp")
        gyp = pstat.tile([128, 128], f32, name="gyp", tag="gyp")
        nc.vector.tensor_tensor(out=gxp[:], in0=xmp[:, 2:130], in1=xmp[:, 0:128], op=ALU.subtract)
        nc.vector.scalar_tensor_tensor(out=gyp[:], in0=xmp[:, 1:129], scalar=2.0,
                                       in1=xmp[:, 0:128], op0=ALU.mult, op1=ALU.add)
        nc.vector.tensor_tensor(out=gyp[:], in0=gyp[:], in1=xmp[:, 2:130], op=ALU.add)
        psx = ppsm.tile([128, 128], f32, name="psx", tag="sm")
        nc.tensor.matmul(psx[:], lhsT=bsbd_s[:, 0:128], rhs=gxp[:], start=True, stop=True)
        psy = ppsm.tile([128, 128], f32, name="psy", tag="sm")
        nc.tensor.matmul(psy[:], lhsT=bsbd_s[:, 128:256], rhs=gyp[:], start=True, stop=True)
        mag = pstat.tile([128, 128], f32, name="mag", tag="mag")
        m2 = pstat.tile([128, 128], f32, name="m2t", tag="m2t")
        nc.scalar.activation(mag[:], psx[:], AF.Square)
        nc.scalar.activation(m2[:], psy[:], AF.Square)
        nc.vector.tensor_tensor(out=mag[:], in0=mag[:], in1=m2[:], op=ALU.add)
        nc.scalar.activation(mag[:], mag[:], AF.Sqrt, bias=c_eps8[:], scale=1.0)
        from concourse import bass_isa
        mxn = pstat.tile([128, 2], f32, name="mxn", tag="mxn")
        nc.vector.tensor_reduce(mxn[:, 0:1], mag[:], axis=AX.X, op=ALU.max)
        nc.vector.tensor_reduce(mxn[:, 1:2], mag[:], axis=AX.X, op=ALU.min,
                                negate=True)
        arx = pstat.tile([128, 2], f32, name="arx", tag="arx")
        nc.gpsimd.partition_all_reduce(arx[:], mxn[:], channels=128,
                                       reduce_op=bass_isa.ReduceOp.max)
        # arx[:,0] = emax, arx[:,1] = -emin  (on every partition)
        den = pstat.tile([128, 1], f32, name="den", tag="den")
        nc.vector.tensor_tensor(out=den[:], in0=arx[:, 0:1], in1=arx[:, 1:2], op=ALU.add)
        nc.vector.tensor_scalar_add(den[:], den[:], 1e-8)
        reca = pstat.tile([128, 1], f32, name="reca", tag="reca")
        nc.vector.reciprocal(reca[:], den[:])
        bea = pstat.tile([128, 1], f32, name="bea", tag="bea")
        nc.vector.tensor_tensor(out=bea[:], in0=arx[:, 1:2], in1=reca[:], op=ALU.mult)
        nc.scalar.activation(edge_f[:], mag[:], AF.Identity, scale=reca[:], bias=bea[:])
        nc.sync.dma_start(edge_o[:, :], edge_f[:])
        nc.vector.tensor_copy(edge_b[:], edge_f[:])
        nc.sync.dma_start(eflat_sp[0:1, :], edge_b[:])

        # ================ P2a: gate conv (uses fz + edge9) ================
        pg1 = tc.alloc_tile_pool(name="pg1", bufs=1)
        g1n = pg1.tile([128, LC], bf16, name="g1n", tag="g1n")
        zero_pads(nc, g1n)

        pe9 = tc.alloc_tile_pool(name="pe9", bufs=1, side="right")
        e9 = pe9.tile([9, LC], bf16, name="e9", tag="e9")
        nc.vector.memset(e9[:], 0.0)
        for t, (ty, tx) in enumerate(TAPS):
            base_t = (1 - ty) * CW + (1 - tx)
            dst = e9[t:t + 1, base_t:base_t + H * CW].rearrange("p (r x) -> p r x", r=H)[:, :, 0:W]
            nc.sync.dma_start(dst, edge_b[:])

        scrg = pstat.tile([128, NPT], f32, name="scrg", tag="scrg")
        for tp in range(0, NT, 2):
            j = tp // 2
            pt = pps.tile([128, NPT], f32, name=f"g1p{tp}", tag="acc")
            for c in range(2):
                for t in range(9):
                    col0 = (t * 2 + c) * 64
                    off = TAPS[t][0] * CW + TAPS[t][1]
                    first = (c == 0 and t == 0)
                    nc.tensor.matmul(pt[0:64, :], lhsT=wg1_s[:, col0:col0 + 64],
                                     rhs=win(fz[c], tp, off),
                                     start=first, stop=False, tile_position=(0, 0),
                                     skip_group_check=True)
                    nc.tensor.matmul(pt[64:128, :], lhsT=wg1_s[:, col0:col0 + 64],
                                     rhs=win(fz[c], tp + 1, off),
                                     start=first, stop=False, tile_position=(0, 64),
                                     skip_group_check=True)
            nc.tensor.matmul(pt[0:64, :], lhsT=wg1e_s[:], rhs=win(e9, tp, 0),
                             start=False, stop=True, tile_position=(0, 0),
                             skip_group_check=True)
            nc.tensor.matmul(pt[64:128, :], lhsT=wg1e_s[:], rhs=win(e9, tp + 1, 0),
                             start=False, stop=True, tile_position=(0, 64),
                             skip_group_check=True)
            wA = win(g1n, tp, 0)[0:64]
            wB = win(g1n, tp + 1, 0)[64:128]
            nc.scalar.activation(wA, pt[0:64, :], AF.Copy,
                                 accum_out=g1sum[0:64, j:j + 1])
            nc.scalar.activation(wB, pt[64:128, :], AF.Copy,
                                 accum_out=g1sum[64:128, j:j + 1])
            nc.scalar.activation(scrg[:], pt[:], AF.Square,
                                 accum_out=g1sq[:, j:j + 1])
        pe9.release()

        # gate GN finalize (col-packed: channel c lives on partitions c and c+64)
        gchan = pstat.tile([128, 2], f32, name="gchan", tag="gchan")
        nc.vector.tensor_reduce(gchan[:, 0:1], g1sum[:], axis=AX.X, op=ALU.add)
        nc.vector.tensor_reduce(gchan[:, 1:2], g1sq[:], axis=AX.X, op=ALU.add)
        ggp = ppsm.tile([64, 2], f32, name="ggp", tag="sm")
        nc.tensor.matmul(ggp[:], lhsT=gmat8_s[:], rhs=gchan[:], start=True, stop=True)
        gnpix = 8 * HW
        gmv = pstat.tile([64, 2], f32, name="gmv", tag="gmv")
        nc.vector.tensor_scalar_mul(gmv[:], ggp[:], 1.0 / gnpix)
        gvar = pstat.tile([64, 1], f32, name="gvar", tag="gvar")
        nc.vector.tensor_tensor(out=gvar[:], in0=gmv[:, 0:1], in1=gmv[:, 0:1], op=ALU.mult)
        nc.vector.tensor_tensor(out=gvar[:], in0=gmv[:, 1:2], in1=gvar[:], op=ALU.subtract)
        gsd = pstat.tile([64, 1], f32, name="gsd", tag="gsd")
        nc.scalar.activation(gsd[:], gvar[:], AF.Sqrt, bias=c_epsgn[0:64, :], scale=1.0)
        grinv = pstat.tile([64, 1], f32, name="grinv", tag="grinv")
        nc.vector.reciprocal(grinv[:], gsd[:])
        agbg = pstat.tile([128, 2], f32, name="agbg", tag="agbg")
        nc.vector.tensor_tensor(out=agbg[0:64, 0:1], in0=gnwg_s[:], in1=grinv[:], op=ALU.mult)
        nc.vector.scalar_tensor_tensor(out=agbg[0:64, 1:2], in0=gmv[:, 0:1], scalar=-1.0,
                                       in1=agbg[0:64, 0:1], op0=ALU.mult, op1=ALU.mult)
        nc.vector.tensor_tensor(out=agbg[0:64, 1:2], in0=agbg[0:64, 1:2], in1=gnbg_s[:], op=ALU.add)
        nc.sync.dma_start(agbg[64:128, :], agbg[0:64, :])
        for ti in range(NT):
            h0, h1_ = (0, 64) if ti % 2 == 0 else (64, 128)
            wT = win(g1n, ti, 0)[h0:h1_]
            nc.scalar.activation(wT, wT, AF.Gelu,
                                 scale=agbg[h0:h1_, 0:1], bias=agbg[h0:h1_, 1:2])

        # gate2 1x1 conv + sigmoid -> gate_o + scaled bf16 gate spill
        pgsb = tc.alloc_tile_pool(name="pgsb", bufs=1)
        gsb = pgsb.tile([1, HW], bf16, name="gsb", tag="gsb")
        for ti in range(NT):
            h0, h1_ = (0, 64) if ti % 2 == 0 else (64, 128)
            pt1 = ppsm.tile([1, NPT], f32, name=f"g2p{ti}", tag="sm")
            nc.tensor.matmul(pt1[:], lhsT=wg2_s[h0:h1_, :], rhs=win(g1n, ti, 0)[h0:h1_],
                             start=True, stop=True)
            s1 = pstage.tile([1, NPT], f32, name=f"g2s{ti}", tag="st1", bufs=2)
            nc.scalar.activation(s1[:], pt1[:], AF.Sigmoid, bias=b2s_s[:], scale=1.0)
            nc.sync.dma_start(gate_o[0:1, ti * NPT:(ti + 1) * NPT], s1[:])
            nc.vector.tensor_scalar(out=gsb[:, ti * NPT:(ti + 1) * NPT], in0=s1[:],
                                    scalar1=rss_s[:], scalar2=None, op0=ALU.mult)
        nc.sync.dma_start(gspill[0:1, :], gsb[:])
        pgsb.release()
        pg1.release()
        pfz.release()

        # ================ P2b: hf1 conv (reload hf canvases) ================
        phf = tc.alloc_tile_pool(name="phf", bufs=1, side="right")
        hfc = [phf.tile([128, LC], bf16, name=f"hfc{c}", tag=f"hfc{c}") for c in range(2)]
        zero_pads(nc, hfc[0])
        zero_pads(nc, hfc[1])
        for c in range(2):
            for ti in range(NT):
                hr = pstage.tile([128, NPT], bf16, name=f"hr{c}_{ti}", tag="stbf", bufs=6)
                nc.sync.dma_start(hr[:], hfsp[c][:, ti * NPT:(ti + 1) * NPT])
                nc.vector.tensor_copy(win(hfc[c], ti, 0),
                                      hr[:].rearrange("p (r x) -> p r x", r=RT))
        ph1 = tc.alloc_tile_pool(name="ph1", bufs=1)
        h1n = ph1.tile([128, LC], bf16, name="h1n", tag="h1n")
        zero_pads(nc, h1n)

        scr128 = pstat.tile([128, NPT], f32, name="scr128", tag="scr128")
        for ti in range(NT):
            pt = pps.tile([128, NPT], f32, name=f"h1p{ti}", tag="acc")
            k = 0
            for c in range(2):
                for t in range(9):
                    col0 = (t * 2 + c) * 128
                    nc.tensor.matmul(pt[:], lhsT=whf1_s[:, col0:col0 + 128],
                                     rhs=win(hfc[c], ti, TAPS[t][0] * CW + TAPS[t][1]),
                                     start=(k == 0), stop=(k == 17))
                    k += 1
            nc.scalar.activation(win(h1n, ti, 0), pt[:], AF.Copy,
                                 accum_out=h1sum[:, ti:ti + 1])
            nc.scalar.activation(scr128[:], pt[:], AF.Square,
                                 accum_out=h1sq[:, ti:ti + 1])
        phf.release()

        ah, bh = gn_finalize(128, 16, [h1sum], [h1sq], gmat16_s, gnwh_s, gnbh_s,
                             16 * HW, "h")
        for ti in range(NT):
            nc.scalar.activation(win(h1n, ti, 0), win(h1n, ti, 0), AF.Gelu,
                                 scale=ah[:, 0:1], bias=bh[:, 0:1])

        # ================ P3: hf2 conv -> rin = fused + edge*hf_feat ================
        prin = tc.alloc_tile_pool(name="prin", bufs=1, side="right")
        rin = [prin.tile([128, LC], bf16, name=f"rin{c}", tag=f"rin{c}") for c in range(2)]
        pedge = tc.alloc_tile_pool(name="pedge", bufs=1, side="right")
        ebc = pedge.tile([128, HW], bf16, name="ebc", tag="ebc")
        pesb = tc.alloc_tile_pool(name="pesb", bufs=2, side="right")
        QB = 1024
        for q in range(HW // QB):
            esb = pesb.tile([1, QB], bf16, name=f"esb{q}", tag="esb")
            nc.sync.dma_start(esb[:], eflat_sp[0:1, q * QB:(q + 1) * QB])
            nc.gpsimd.partition_broadcast(ebc[:, q * QB:(q + 1) * QB], esb[:])
        pesb.release()
        zero_pads(nc, rin[0])
        zero_pads(nc, rin[1])

        for ti in range(NT):
            for m in range(2):
                pt = pps.tile([128, NPT], f32, name=f"h2p{ti}_{m}", tag="acc")
                for t in range(9):
                    col0 = (t * 2 + m) * 128
                    nc.tensor.matmul(pt[:], lhsT=whf2_s[:, col0:col0 + 128],
                                     rhs=win(h1n, ti, TAPS[t][0] * CW + TAPS[t][1]),
                                     start=(t == 0), stop=(t == 8))
                t1 = pstage.tile([128, NPT], f32, name=f"h2t{ti}_{m}", tag="stf32b", bufs=2)
                nc.vector.tensor_tensor(out=t1[:], in0=pt[:],
                                        in1=ebc[:, ti * NPT:(ti + 1) * NPT], op=ALU.mult)
                fb = pstage.tile([128, NPT], bf16, name=f"h2f{ti}_{m}", tag="stbf", bufs=6)
                nc.sync.dma_start(fb[:], fzlin[m][:, ti * NPT:(ti + 1) * NPT])
                nc.vector.tensor_tensor(out=win(rin[m], ti, 0), in0=t1[:], in1=fb[:], op=ALU.add)
        ph1.release()
        pedge.release()

        # ================ P4: ref1 conv ================
        pr1 = tc.alloc_tile_pool(name="pr1", bufs=1)
        r1n = [pr1.tile([128, LC], bf16, name=f"r1n{c}", tag=f"r1n{c}") for c in range(2)]
        zero_pads(nc, r1n[0])
        zero_pads(nc, r1n[1])

        for ti in range(NT):
            for m in range(2):
                pt = pps.tile([128, NPT], f32, name=f"r1p{ti}_{m}", tag="acc")
                k = 0
                for c in range(2):
                    for t in range(9):
                        col0 = ((t * 2 + c) * 2 + m) * 128
                        nc.tensor.matmul(pt[:], lhsT=wr1_s[:, col0:col0 + 128],
                                         rhs=win(rin[c], ti, TAPS[t][0] * CW + TAPS[t][1]),
                                         start=(k == 0), stop=(k == 17))
                        k += 1
                nc.scalar.activation(win(r1n[m], ti, 0), pt[:], AF.Copy,
                                     accum_out=r1sum[m][:, ti:ti + 1])
                nc.scalar.activation(scr128[:], pt[:], AF.Square,
                                     accum_out=r1sq[m][:, ti:ti + 1])
        prin.release()

        ar, br = gn_finalize(128, 32, r1sum, r1sq, gmat32_s, gnwr_s, gnbr_s,
                             32 * HW, "r")
        for ti in range(NT):
            for m in range(2):
                nc.scalar.activation(win(r1n[m], ti, 0), win(r1n[m], ti, 0), AF.Gelu,
                                     scale=ar[:, m:m + 1], bias=br[:, m:m + 1])

        # ================ P5: ref2 conv -> refined ================
        pgb = tc.alloc_tile_pool(name="pgb", bufs=1)
        gbc = pgb.tile([128, HW], bf16, name="gbc", tag="gbc")
        pgsb2 = tc.alloc_tile_pool(name="pgsb2", bufs=2)
        QB = 1024
        for q in range(HW // QB):
            gsb2 = pgsb2.tile([1, QB], bf16, name=f"gsb2{q}", tag="gsb2")
            nc.sync.dma_start(gsb2[:], gspill[0:1, q * QB:(q + 1) * QB])
            nc.gpsimd.partition_broadcast(gbc[:, q * QB:(q + 1) * QB], gsb2[:])
        pgsb2.release()

        for ti in range(NT):
            for m in range(2):
                pt = pps.tile([128, NPT], f32, name=f"r2p{ti}_{m}", tag="acc")
                k = 0
                for c in range(2):
                    for t in range(9):
                        col0 = ((t * 2 + c) * 2 + m) * 128
                        nc.tensor.matmul(pt[:], lhsT=wr2_s[:, col0:col0 + 128],
                                         rhs=win(r1n[c], ti, TAPS[t][0] * CW + TAPS[t][1]),
                                         start=(k == 0), stop=(k == 17))
                        k += 1
                t1 = pstage.tile([128, NPT], f32, name=f"r2t{ti}_{m}", tag="stf32b", bufs=2)
                nc.vector.tensor_tensor(out=t1[:], in0=pt[:],
                                        in1=gbc[:, ti * NPT:(ti + 1) * NPT], op=ALU.mult)
                ff = pstage.tile([128, NPT], f32, name=f"r2f{ti}_{m}", tag="stf32", bufs=6)
                nc.sync.dma_start(ff[:], fused[m * 128:(m + 1) * 128, ti * NPT:(ti + 1) * NPT])
                ot = pstage.tile([128, NPT], f32, name=f"r2o{ti}_{m}", tag="stf32c", bufs=2)
                nc.vector.tensor_tensor(out=ot[:], in0=t1[:], in1=ff[:], op=ALU.add)
                nc.sync.dma_start(refined[m * 128:(m + 1) * 128, ti * NPT:(ti + 1) * NPT], ot[:])
        pgb.release()
        pr1.release()

    nc.compile()
    _CACHE["built"] = nc
    return nc


# ---------------- host orchestration ----------------
def _prep_inputs(inputs):
    f32 = np.float32
    consts = _host_consts()
    shared = {
        "whf1": _pack_conv(np.asarray(inputs["hf_w1"], f32), 2, 1, 128),
        "whf2": _pack_conv(np.asarray(inputs["hf_w2"], f32), 1, 2, 128),
        "wg1": _pack_conv(np.asarray(inputs["gate_w1"], f32)[:, :256], 2, 1, 64),
        "wg1e": np.ascontiguousarray(
            np.asarray(inputs["gate_w1"], f32)[:, 256, :, :].reshape(64, 9).T
        ).astype(ml_dtypes.bfloat16),
        "wg2": np.tile(np.asarray(inputs["gate_w2"], f32).reshape(1, 64).T, (2, 1)).astype(ml_dtypes.bfloat16).copy(),
        "wr1": _pack_conv(np.asarray(inputs["ref_w1"], f32), 2, 2, 128),
        "wr2": _pack_conv(np.asarray(inputs["ref_w2"], f32), 2, 2, 128),
        "bsbd": consts["bsbd"],
        "gmat16": consts["gmat16"], "gmat32": consts["gmat32"], "gmat8": consts["gmat8"],
        "idlap": consts["idlap"], "ones1": consts["ones1"],
        "gnwh": np.asarray(inputs["hf_gn_w"], f32).reshape(128, 1),
        "gnbh": np.asarray(inputs["hf_gn_b"], f32).reshape(128, 1),
        "gnwg": np.asarray(inputs["gate_gn_w"], f32).reshape(64, 1),
        "gnbg": np.asarray(inputs["gate_gn_b"], f32).reshape(64, 1),
        "gnwr": np.asarray(inputs["ref_gn_w"], f32).reshape(2, 128).T.copy(),
        "gnbr": np.asarray(inputs["ref_gn_b"], f32).reshape(2, 128).T.copy(),
        "b2s": np.asarray(inputs["gate_b2"], f32).reshape(1, 1),
        "rss": np.asarray(inputs["residual_scale"], f32).reshape(1, 1),
    }
    fused = np.asarray(inputs["fused"], f32)
    per_core = []
    for b in range(8):
        m = dict(shared)
        m["fused"] = np.ascontiguousarray(fused[b].reshape(256, HW))
        per_core.append(m)
    return per_core


def _postprocess(results):
    refined = np.stack([r["refined"].reshape(256, H, W) for r in results]).astype(np.float32)
    edge = np.stack([r["edge_o"].reshape(1, H, W) for r in results]).astype(np.float32)
    gate = np.stack([r["gate_o"].reshape(-1) for r in results])
    gate_mean = np.float32(np.mean(gate.astype(np.float64)))
    return refined, edge, gate_mean


def kernel(**inputs):
    nc = _build()
    per_core = _prep_inputs(inputs)
    from concourse.bass_utils import run_bass_kernel_spmd
    res = run_bass_kernel_spmd(nc, per_core, list(range(8)))
    return _postprocess(res.results)


# ---------------- single-core simulator check (used by test.py) ----------------
def sim_one_core(inputs, core=0):
    import concourse.bass_interp as bass_interp
    nc = _build()
    per_core = _prep_inputs(inputs)
    sim = bass_interp.CoreSim(nc)
    for k, v in per_core[core].items():
        sim.tensor(k)[:] = v
    sim.simulate()
    out = {k: np.array(sim.tensor(k)) for k in ("refined", "edge_o", "gate_o")}
    return out
